# revision 6
# baseline (speedup 1.0000x reference)

# Trainium2 Bass kernel for nn_DRRGHead (1x1 conv head + 4-layer GraphConv GCN
# over 512 independent local graphs + knn-gather classifier tail).
#
# Sharding (8 cores, data-parallel):
#   - image: H=512 split into 8 slabs of 64 rows; each core convolves
#     (4 batches x 32 ch x 64 rows x 512 cols) with the 6x32 1x1-conv.
#   - graphs: G=512 split into 64 graphs/core, processed as 22 blocks of
#     3 graphs (3*41=123 rows on partitions; last block 1 graph, zero-padded).
#
# BatchNorm handling: BN(x) = x*s + t is affine and A is row-normalized
# (A @ 1 == 1), so BN folds into layer-1 weights/bias:
#   [BN(x), A BN(x)] @ W1 + b1 == [x, A x] @ (s_dup*W1) + (b1 + t@(W1a+W1b)).
# A small first dispatch computes per-core partial sums/sumsq of node_feats
# (the full 48-MB reduction stays on device); the host only adds the eight
# partial [640,2] vectors, forms s,t and rescales W1 (0.3 MFLOP of glue).
#
# Layout scheme on device (per block of 123 rows):
#   x_l kept rows-on-partitions ("L_R").  One PE matmul per feature-chunk
#   with rhs = [blockdiag(A_g^T) | I_123] yields [agg_l^T | x_l^T]
#   (features-on-partitions, "L_F") directly -- the transpose needed by the
#   next dense layer falls out of the same matmul, so the whole GCN chain
#   needs zero explicit transposes.  Dense layers contract features:
#   lhsT = catT chunks (L_F), rhs = W chunks -> x_{l+1} in L_R.  Layer 4
#   instead uses lhsT = W4 chunks (L_F out) because the tail wants x4^T.
#   Bias is added with a rank-1 matmul (ones[1,123] x b[1,outs]) into PSUM,
#   ReLU applied on PSUM->SBUF eviction.
#
# Conv: lhsT = blockdiag over 4 batches of conv_w^T (128x24), rhs = pixel
# chunks [128, 512]; 4 chunks run concurrently in distinct PE col-strips via
# tile_position=(0,32j), evicted 96-lanes-wide with per-partition bias.

import os
import sys
import threading

import numpy as np

if "/opt/trn_rl_repo" not in sys.path:
    sys.path.insert(0, "/opt/trn_rl_repo")

import concourse.bass as bass  # noqa: F401
import concourse.tile as tile
from concourse import bacc, mybir

FP32 = mybir.dt.float32
AF = mybir.ActivationFunctionType

N_CORES = 8
B, C, H, W = 4, 32, 512, 512
G, N, F = 512, 41, 576
CO = 6                     # conv out channels
HS = H // N_CORES          # 64 image rows per core
NPX = HS * W               # 32768 pixels per (core, batch)
GC = G // N_CORES          # 64 graphs per core
GPB = 3                    # graphs per block
NB = 123                   # rows per block (3*41)
NBLK = 22                  # blocks per core (21 full + 1 single-graph)
RPAD = NBLK * NB           # 2706 padded rows per core
ATW = 256                  # per-block [blockdiag(A^T) | I | 0pad] width
FPAD = 640                 # 576 padded to 5*128
EPS = 1e-5

_lock = threading.Lock()
_cache = {}


# ----------------------------------------------------------------- builders
def _build_stats_nc():
    """Per-core partial sum / sum-of-squares of node_feats over rows.

    Input  nft  [FPAD, RPAD]  (features on partitions, zero-padded)
    Output stats [FPAD, 2]    (col 0 = sum, col 1 = sumsq)
    """
    nc = bacc.Bacc("TRN2", target_bir_lowering=False, debug=False,
                   num_devices=N_CORES)
    nft = nc.dram_tensor("nft", [FPAD, RPAD], FP32, kind="ExternalInput").ap()
    stats = nc.dram_tensor("stats", [FPAD, 2], FP32, kind="ExternalOutput").ap()
    with tile.TileContext(nc) as tc:
        with (
            tc.tile_pool(name="io", bufs=2) as io,
            tc.tile_pool(name="scratch", bufs=2) as sp,
            tc.tile_pool(name="acc", bufs=10) as ap,
        ):
            for k in range(FPAD // 128):
                t = io.tile([128, RPAD], FP32)
                nc.sync.dma_start(t[:], nft[128 * k:128 * (k + 1), :])
                sums = ap.tile([128, 1], FP32, tag="sums")
                nc.vector.reduce_sum(sums[:], t[:], axis=mybir.AxisListType.X)
                sq = sp.tile([128, RPAD], FP32)
                sqs = ap.tile([128, 1], FP32, tag="sqs")
                nc.scalar.activation(sq[:], t[:], AF.Square, accum_out=sqs[:])
                nc.sync.dma_start(stats[128 * k:128 * (k + 1), 0:1], sums[:])
                nc.sync.dma_start(stats[128 * k:128 * (k + 1), 1:2], sqs[:])
    nc.compile()
    return nc


def _build_main_nc(reps=1):
    nc = bacc.Bacc("TRN2", target_bir_lowering=False, debug=False,
                   num_devices=N_CORES)
    dt = nc.dram_tensor
    aps = {}

    def di(name, shape):
        aps[name] = dt(name, shape, FP32, kind="ExternalInput").ap()

    di("img", [128, NPX])
    di("x0r", [NB, NBLK * FPAD])
    di("nft", [FPAD, RPAD])
    di("atbi", [NB, NBLK * ATW])
    di("w1x", [FPAD, 512]); di("w1a", [FPAD, 512])
    di("w2x", [512, 256]); di("w2a", [512, 256])
    di("w3x", [256, 128]); di("w3a", [256, 128])
    di("w4x", [128, 64]); di("w4a", [128, 64])
    di("b1", [1, 512]); di("b2", [1, 256]); di("b3", [1, 128])
    di("b4", [64, 1])
    di("cw1", [64, 32]); di("cb1", [32, 1]); di("pa", [32, 1])
    di("cw2", [32, 2]); di("cb2", [2, 1])
    di("cwbd", [128, 32]); di("cb128", [128, 1])
    aps["pred"] = dt("pred", [B, CO, HS, W], FP32, kind="ExternalOutput").ap()
    aps["predt"] = dt("predt", [2, RPAD], FP32, kind="ExternalOutput").ap()

    with tile.TileContext(nc) as tc:
        _emit_main(nc, tc, aps, reps)
    nc.compile()
    return nc


def _emit_main(nc, tc, t, reps):
    from contextlib import ExitStack
    ctx = ExitStack()
    with ctx:
        def pool(name, bufs, space="SBUF"):
            return ctx.enter_context(
                tc.tile_pool(name=name, bufs=bufs, space=space))

        consts = pool("consts", 1)
        imgp = pool("imgp", 3)
        cop = pool("cop", 2)
        x0p = pool("x0p", 2)
        atp = pool("atp", 2)
        blkp = pool("blk", 2)
        x4p = pool("x4p", 1)
        predp = pool("predp", 1)
        hp = pool("hp", 2)
        psc = pool("psc", 2, space="PSUM")    # conv psum [128,512]
        psb = pool("psb", 3, space="PSUM")    # block psum, slots [128,1024]

        # ---- resident constants / weights -------------------------------
        def load_const(ap_in, shape, tag):
            s = consts.tile(shape, FP32, tag=tag)
            nc.sync.dma_start(s[:], ap_in)
            return s

        def load_chunks(name, nchunk, width):
            return [load_const(t[name][128 * k:128 * (k + 1), :],
                               [128, width], f"{name}{k}")
                    for k in range(nchunk)]

        w1x_s = load_chunks("w1x", 5, 512)
        w1a_s = load_chunks("w1a", 5, 512)
        w2x_s = load_chunks("w2x", 4, 256)
        w2a_s = load_chunks("w2a", 4, 256)
        w3x_s = load_chunks("w3x", 2, 128)
        w3a_s = load_chunks("w3a", 2, 128)
        w4x_s = load_const(t["w4x"][:], [128, 64], "w4x")
        w4a_s = load_const(t["w4a"][:], [128, 64], "w4a")
        b1_s = load_const(t["b1"][:], [1, 512], "b1")
        b2_s = load_const(t["b2"][:], [1, 256], "b2")
        b3_s = load_const(t["b3"][:], [1, 128], "b3")
        b4_s = load_const(t["b4"][:], [64, 1], "b4")
        cw1_s = load_const(t["cw1"][:], [64, 32], "cw1")
        cb1_s = load_const(t["cb1"][:], [32, 1], "cb1")
        pa_s = load_const(t["pa"][:], [32, 1], "pa")
        cw2_s = load_const(t["cw2"][:], [32, 2], "cw2")
        cb2_s = load_const(t["cb2"][:], [2, 1], "cb2")
        cwbd_s = load_const(t["cwbd"][:], [128, 32], "cwbd")
        cb128_s = load_const(t["cb128"][:], [128, 1], "cb128")
        nft_s = load_chunks("nft", 5, RPAD)
        ones_s = consts.tile([1, NB], FP32, tag="ones")
        nc.vector.memset(ones_s[:], 1.0)

        # dest view for conv DMA: h split as (quad, s, j)
        pred_v = t["pred"].rearrange("b o (q s j) w -> q j (b o) s w",
                                     q=4, s=4, j=4)

        for _ in range(reps):
            # ---- conv over this core's image slab -----------------------
            for quad in range(4):
                out_sb = cop.tile([128, 4 * 512], FP32, tag="convout")
                for s in range(4):
                    sr = 4 * quad + s            # super-round 0..15
                    it = imgp.tile([128, 2048], FP32, tag="img")
                    nc.sync.dma_start(
                        it[:], t["img"][:, 2048 * sr:2048 * (sr + 1)])
                    ps = psc.tile([128, 512], FP32, tag="convps")
                    for j in range(4):
                        nc.tensor.matmul(
                            ps[32 * j:32 * (j + 1), :],
                            cwbd_s[:, 0:32],
                            it[:, 512 * j:512 * (j + 1)],
                            start=True, stop=True,
                            tile_position=(0, 32 * j),
                        )
                    nc.scalar.activation(out_sb[:, 512 * s:512 * (s + 1)],
                                         ps[:], AF.Identity, bias=cb128_s[:])
                osv = out_sb.rearrange("(j x) (s w) -> j x s w", j=4, s=4)
                for j in range(4):
                    nc.sync.dma_start(pred_v[quad, j], osv[j, 0:24])

            # ---- GCN block chain ---------------------------------------
            x4t_all = x4p.tile([64, RPAD], FP32, tag="x4t")
            for b in range(NBLK):
                x0_b = x0p.tile([NB, FPAD], FP32, tag="x0")
                nc.sync.dma_start(
                    x0_b[:], t["x0r"][:, FPAD * b:FPAD * (b + 1)])
                at_b = atp.tile([NB, ATW], FP32, tag="at")
                nc.sync.dma_start(
                    at_b[:], t["atbi"][:, ATW * b:ATW * (b + 1)])

                # bmm1: agg1T chunks [128,123] at free offset 128*fc
                ps_c1 = psb.tile([128, 640], FP32, tag="ps")
                for fc in range(5):
                    nc.tensor.matmul(
                        ps_c1[:, 128 * fc:128 * (fc + 1)],
                        x0_b[:, 128 * fc:128 * (fc + 1)],
                        at_b[:, 0:128], start=True, stop=True)
                c1 = blkp.tile([128, 640], FP32, tag="c1")
                nc.vector.tensor_copy(c1[:], ps_c1[:])

                # dense1 (option A): x1 = relu([x0,agg1] @ W1' + b1')
                ps_x1 = psb.tile([NB, 512], FP32, tag="ps")
                for k in range(5):
                    nc.tensor.matmul(
                        ps_x1[:], nft_s[k][:, NB * b:NB * (b + 1)],
                        w1x_s[k][:], start=(k == 0), stop=False)
                for k in range(5):
                    nc.tensor.matmul(
                        ps_x1[:], c1[:, 128 * k:128 * k + NB],
                        w1a_s[k][:], start=False, stop=False)
                nc.tensor.matmul(ps_x1[:], ones_s[:], b1_s[:],
                                 start=False, stop=True)
                x1 = blkp.tile([NB, 512], FP32, tag="x1")
                nc.scalar.activation(x1[:], ps_x1[:], AF.Relu)

                # bmm2 combined: [agg2T | x1T] chunks at 256*fc
                ps_c2 = psb.tile([128, 1024], FP32, tag="ps")
                for fc in range(4):
                    nc.tensor.matmul(
                        ps_c2[:, 256 * fc:256 * (fc + 1)],
                        x1[:, 128 * fc:128 * (fc + 1)],
                        at_b[:], start=True, stop=True)
                c2 = blkp.tile([128, 1024], FP32, tag="c2")
                nc.scalar.copy(c2[:], ps_c2[:])

                ps_x2 = psb.tile([NB, 256], FP32, tag="ps")
                for k in range(4):
                    nc.tensor.matmul(
                        ps_x2[:], c2[:, 256 * k + NB:256 * k + 2 * NB],
                        w2x_s[k][:], start=(k == 0), stop=False)
                for k in range(4):
                    nc.tensor.matmul(
                        ps_x2[:], c2[:, 256 * k:256 * k + NB],
                        w2a_s[k][:], start=False, stop=False)
                nc.tensor.matmul(ps_x2[:], ones_s[:], b2_s[:],
                                 start=False, stop=True)
                x2 = blkp.tile([NB, 256], FP32, tag="x2")
                nc.vector.tensor_scalar_max(x2[:], ps_x2[:], 0.0)

                # bmm3
                ps_c3 = psb.tile([128, 512], FP32, tag="ps")
                for fc in range(2):
                    nc.tensor.matmul(
                        ps_c3[:, 256 * fc:256 * (fc + 1)],
                        x2[:, 128 * fc:128 * (fc + 1)],
                        at_b[:], start=True, stop=True)
                c3 = blkp.tile([128, 512], FP32, tag="c3")
                nc.vector.tensor_copy(c3[:], ps_c3[:])

                ps_x3 = psb.tile([NB, 128], FP32, tag="ps")
                for k in range(2):
                    nc.tensor.matmul(
                        ps_x3[:], c3[:, 256 * k + NB:256 * k + 2 * NB],
                        w3x_s[k][:], start=(k == 0), stop=False)
                for k in range(2):
                    nc.tensor.matmul(
                        ps_x3[:], c3[:, 256 * k:256 * k + NB],
                        w3a_s[k][:], start=False, stop=False)
                nc.tensor.matmul(ps_x3[:], ones_s[:], b3_s[:],
                                 start=False, stop=True)
                x3 = blkp.tile([NB, 128], FP32, tag="x3")
                nc.scalar.activation(x3[:], ps_x3[:], AF.Relu)

                # bmm4
                ps_c4 = psb.tile([128, ATW], FP32, tag="ps")
                nc.tensor.matmul(ps_c4[:], x3[:], at_b[:],
                                 start=True, stop=True)
                c4 = blkp.tile([128, ATW], FP32, tag="c4")
                nc.vector.tensor_copy(c4[:], ps_c4[:])

                # dense4 (option B): x4T = relu(W4.T catT + b4)
                ps_x4 = psb.tile([64, NB], FP32, tag="ps")
                nc.tensor.matmul(ps_x4[:], w4x_s[:], c4[:, NB:2 * NB],
                                 start=True, stop=False)
                nc.tensor.matmul(ps_x4[:], w4a_s[:], c4[:, 0:NB],
                                 start=False, stop=True)
                nc.scalar.activation(x4t_all[:, NB * b:NB * (b + 1)],
                                     ps_x4[:], AF.Relu, bias=b4_s[:])

            # ---- classifier tail on x4T --------------------------------
            predt_sb = predp.tile([2, RPAD], FP32, tag="predt")
            CH = [512] * 5 + [RPAD - 5 * 512]
            off = 0
            for w in CH:
                ps_h = psb.tile([32, 512], FP32, tag="ps")
                nc.tensor.matmul(ps_h[:, 0:w], cw1_s[:],
                                 x4t_all[:, off:off + w],
                                 start=True, stop=True)
                # PReLU(z) = max(z,0) + a*min(z,0), z = W1h@x4 + b1h
                z_sb = hp.tile([32, 512], FP32, tag="z")
                nc.scalar.activation(z_sb[:, 0:w], ps_h[:, 0:w], AF.Identity,
                                     bias=cb1_s[:])
                hneg = hp.tile([32, 512], FP32, tag="hneg")
                nc.vector.tensor_scalar(hneg[:, 0:w], z_sb[:, 0:w], 0.0,
                                        pa_s[:], mybir.AluOpType.min,
                                        mybir.AluOpType.mult)
                h_sb = hp.tile([32, 512], FP32, tag="h")
                nc.vector.tensor_scalar_max(h_sb[:, 0:w], z_sb[:, 0:w], 0.0)
                nc.vector.tensor_add(h_sb[:, 0:w], h_sb[:, 0:w],
                                     hneg[:, 0:w])
                ps_p = psb.tile([2, 512], FP32, tag="ps")
                nc.tensor.matmul(ps_p[:, 0:w], cw2_s[:], h_sb[:, 0:w],
                                 start=True, stop=True)
                nc.scalar.activation(predt_sb[:, off:off + w], ps_p[:, 0:w],
                                     AF.Identity, bias=cb2_s[:])
                off += w
            nc.sync.dma_start(t["predt"][:], predt_sb[:])


# ------------------------------------------------------------- host prep
def _pad_rows(sh):
    """[GC*N, F] -> [RPAD, F] with zero pad rows per 3-graph block."""
    rp = np.zeros((RPAD, F), np.float32)
    for b in range(NBLK):
        r0 = b * GPB * N
        rows = min(GPB * N, GC * N - r0)
        rp[b * NB:b * NB + rows] = sh[r0:r0 + rows]
    return rp


def _prep_stats_inputs(node_feats):
    nf = np.ascontiguousarray(node_feats, np.float32).reshape(G, N, F)
    maps = []
    for c in range(N_CORES):
        rp = _pad_rows(nf[c * GC:(c + 1) * GC].reshape(GC * N, F))
        nft = np.zeros((FPAD, RPAD), np.float32)
        nft[:F, :] = rp.T
        maps.append({"nft": nft})
    return maps


def _prep_main_inputs(inputs, node_feats, A, conv_w, conv_b, w1s, b1s,
                      W2, b2v, W3, b3v, W4, b4v,
                      cls_w1, cls_b1, prelu_a, cls_w2, cls_b2):
    f32 = lambda x: np.ascontiguousarray(x, np.float32)
    inputs, node_feats, A = f32(inputs), f32(node_feats), f32(A)
    cwbd = np.zeros((128, 32), np.float32)
    cb128 = np.zeros((128, 1), np.float32)
    wt = f32(conv_w).T  # [32, 6]
    for b in range(B):
        cwbd[32 * b:32 * (b + 1), 6 * b:6 * (b + 1)] = wt
    for j in range(4):
        for b in range(B):
            cb128[32 * j + 6 * b:32 * j + 6 * (b + 1), 0] = f32(conv_b)
    shared = {
        "w1x": np.zeros((FPAD, 512), np.float32),
        "w1a": np.zeros((FPAD, 512), np.float32),
        "w2x": f32(W2[:512]), "w2a": f32(W2[512:]),
        "w3x": f32(W3[:256]), "w3a": f32(W3[256:]),
        "w4x": f32(W4[:128]), "w4a": f32(W4[128:]),
        "b1": f32(b1s).reshape(1, 512), "b2": f32(b2v).reshape(1, 256),
        "b3": f32(b3v).reshape(1, 128), "b4": f32(b4v).reshape(64, 1),
        "cw1": f32(cls_w1), "cb1": f32(cls_b1).reshape(32, 1),
        "pa": f32(prelu_a).reshape(32, 1), "cw2": f32(cls_w2),
        "cb2": f32(cls_b2).reshape(2, 1),
        "cwbd": cwbd, "cb128": cb128,
    }
    shared["w1x"][:F] = w1s[:F]
    shared["w1a"][:F] = w1s[F:]
    eye = np.eye(NB, dtype=np.float32)
    maps = []
    for c in range(N_CORES):
        img = inputs[:, :, c * HS:(c + 1) * HS, :].reshape(128, NPX)
        rp = _pad_rows(node_feats[c * GC:(c + 1) * GC].reshape(GC * N, F))
        x0r = np.zeros((NB, NBLK * FPAD), np.float32)
        x0r_v = rp.reshape(NBLK, NB, F)
        for b in range(NBLK):
            x0r[:, FPAD * b:FPAD * b + F] = x0r_v[b]
        nft = np.zeros((FPAD, RPAD), np.float32)
        nft[:F, :] = rp.T
        atbi = np.zeros((NB, NBLK * ATW), np.float32)
        Ash = A[c * GC:(c + 1) * GC]
        for b in range(NBLK):
            blk = np.zeros((NB, NB), np.float32)
            for gi in range(min(GPB, GC - b * GPB)):
                g = b * GPB + gi
                blk[N * gi:N * (gi + 1), N * gi:N * (gi + 1)] = Ash[g].T
            atbi[:, ATW * b:ATW * b + NB] = blk
            atbi[:, ATW * b + NB:ATW * b + 2 * NB] = eye
        m = dict(shared)
        m.update({"img": np.ascontiguousarray(img), "x0r": x0r, "nft": nft,
                  "atbi": atbi})
        maps.append(m)
    return maps


# ------------------------------------------------------------- execution
def _output_names(nc):
    names = []
    for alloc in nc.m.functions[0].allocations:
        if isinstance(alloc, mybir.MemoryLocationSet) and \
                alloc.kind == "ExternalOutput":
            names.append(alloc.memorylocations[0].name)
    return names


def _run(nc, in_maps):
    if os.environ.get("KERNEL_SIM"):
        from concourse.bass_interp import MultiCoreSim
        sim = MultiCoreSim(nc, num_cores=N_CORES, trace=False)
        for c in range(N_CORES):
            for k, v in in_maps[c].items():
                sim.cores[c].tensor(k)[:] = v
        sim.simulate()
        onames = _output_names(nc)
        return [{n: np.array(sim.cores[c].tensor(n)) for n in onames}
                for c in range(N_CORES)]
    from concourse.bass_utils import run_bass_kernel_spmd
    return run_bass_kernel_spmd(nc, in_maps, list(range(N_CORES))).results


def _get_nc(which, reps=1):
    key = (which, reps)
    with _lock:
        if key not in _cache:
            if which == "stats":
                _cache[key] = _build_stats_nc()
            else:
                _cache[key] = _build_main_nc(reps)
        return _cache[key]


def kernel(inputs, node_feats, A, knn_inx, conv_w, conv_b, W1, b1, W2, b2,
           W3, b3, W4, b4, cls_w1, cls_b1, prelu_a, cls_w2, cls_b2):
    f32 = lambda x: np.asarray(x, np.float32)
    # ---- dispatch A: BN statistics ------------------------------------
    nc_s = _get_nc("stats")
    smaps = _prep_stats_inputs(node_feats)
    sres = _run(nc_s, smaps)
    tot = np.zeros((FPAD, 2), np.float64)
    for c in range(N_CORES):
        tot += sres[c]["stats"].astype(np.float64)
    cnt = float(G * N)
    mean = (tot[:F, 0] / cnt).astype(np.float32)
    var = (tot[:F, 1] / cnt).astype(np.float32) - mean * mean
    s = (1.0 / np.sqrt(var + np.float32(EPS))).astype(np.float32)
    t_ = (-mean * s).astype(np.float32)
    W1 = f32(W1)
    sdup = np.concatenate([s, s]).astype(np.float32)
    w1s = W1 * sdup[:, None]
    b1s = f32(b1) + t_ @ W1[:F] + t_ @ W1[F:]

    # ---- dispatch B: conv + GCN ---------------------------------------
    nc_m = _get_nc("main")
    mmaps = _prep_main_inputs(inputs, node_feats, A, conv_w, conv_b,
                              w1s, b1s, W2, b2, W3, b3, W4, b4,
                              cls_w1, cls_b1, prelu_a, cls_w2, cls_b2)
    mres = _run(nc_m, mmaps)

    # ---- host gather/unshard ------------------------------------------
    pred_maps = np.concatenate(
        [mres[c]["pred"] for c in range(N_CORES)], axis=2)
    x4pred = np.zeros((G * N, 2), np.float32)
    for c in range(N_CORES):
        pt = mres[c]["predt"]  # [2, RPAD]
        sh = pt.T.reshape(NBLK, NB, 2)
        rows = []
        for b in range(NBLK):
            nrows = min(GPB * N, GC * N - b * GPB * N)
            rows.append(sh[b, :nrows])
        x4pred[c * GC * N:(c + 1) * GC * N] = np.concatenate(rows, 0)
    x4pred = x4pred.reshape(G, N, 2)
    ki = np.asarray(knn_inx)
    gcn_pred = np.take_along_axis(
        x4pred, ki[:, :, None].astype(np.int64), axis=1).reshape(-1, 2)
    return pred_maps, gcn_pred


# revision 12
# speedup vs baseline: 228.5517x; 228.5517x over previous

# Trainium2 Bass kernel for nn_DRRGHead (1x1 conv head + 4-layer GraphConv GCN
# over 512 independent local graphs + knn-gather classifier tail).
#
# Sharding (8 cores, data-parallel):
#   - image: H=512 split into 8 slabs of 64 rows; each core convolves
#     (4 batches x 32 ch x 64 rows x 512 cols) with the 6x32 1x1-conv.
#   - graphs: G=512 split into 64 graphs/core, processed as 22 blocks of
#     3 graphs (3*41=123 rows on partitions; last block 1 graph, zero-padded).
#
# BatchNorm handling: BN(x) = x*s + t is affine and A is row-normalized
# (A @ 1 == 1), so BN folds into layer-1 weights/bias:
#   [BN(x), A BN(x)] @ W1 + b1 == [x, A x] @ (s_dup*W1) + (b1 + t@(W1a+W1b)).
# A small first dispatch computes per-core partial sums/sumsq of node_feats
# (the full 48-MB reduction stays on device); the host only adds the eight
# partial [640,2] vectors, forms s,t and rescales W1 (0.3 MFLOP of glue).
#
# Layout scheme on device (per block of 123 rows):
#   x_l kept rows-on-partitions ("L_R").  One PE matmul per feature-chunk
#   with rhs = [blockdiag(A_g^T) | I_123 | 0] (256 wide) yields
#   [agg_l^T | x_l^T] (features-on-partitions, "L_F") directly -- the
#   transpose needed by the next dense layer falls out of the same matmul,
#   so the whole GCN chain needs zero explicit transposes.  Dense layers
#   contract features: lhsT = catT chunks (L_F), rhs = W chunks -> x_{l+1}
#   in L_R.  Layer 4 instead uses lhsT = W4 chunks and batches 4 blocks per
#   matmul (N=492) because the tail wants x4^T.  Bias is added with a
#   rank-1 matmul (ones[1,123] x b[1,outs]) into PSUM; ReLU on eviction.
#
# Matmul dtype: float32r (fp32 container, ~13-bit mantissa used by the PE
# at 1 cycle/row for free-dim >= 256, vs 4 cycles/row for plain fp32).
# HW-verified: f32r matmul is exact on pre-rounded inputs; intermediate
# evictions to f32r tiles round once (~1e-4 rel).  End-to-end impact is
# ~2e-4 on gcn_pred; plain-fp32 fallback via KERNEL_FP32=1.
#
# Conv: lhsT = blockdiag over 4 batches of conv_w^T (128x32, cols 24..31
# zero), rhs = pixel chunks [128, 512]; 4 chunks run concurrently in
# distinct PE col-strips via tile_position=(0,32j), evicted 128-lanes-wide
# with per-partition bias.

import os
import sys
import threading

import numpy as np

if "/opt/trn_rl_repo" not in sys.path:
    sys.path.insert(0, "/opt/trn_rl_repo")

import concourse.bass as bass  # noqa: F401
import concourse.tile as tile
from concourse import bacc, mybir

FP32 = mybir.dt.float32
F32R = mybir.dt.float32r
AF = mybir.ActivationFunctionType

USE_F32R = not os.environ.get("KERNEL_FP32")
MMDT = F32R if USE_F32R else FP32

N_CORES = 8
B, C, H, W = 4, 32, 512, 512
G, N, F = 512, 41, 576
CO = 6                     # conv out channels
HS = H // N_CORES          # 64 image rows per core
NPX = HS * W               # 32768 pixels per (core, batch)
GC = G // N_CORES          # 64 graphs per core
GPB = 3                    # graphs per block
NB = 123                   # rows per block (3*41)
NBLK = 22                  # blocks per core (21 full + 1 single-graph)
RPAD = NBLK * NB           # 2706 padded rows per core
ATW = 256                  # per-block [blockdiag(A^T) | I | 0pad] width
FPAD = 640                 # 576 padded to 5*128
EPS = 1e-5
DGRP = [4, 4, 4, 4, 4, 2]  # dense-4 block batching

_lock = threading.Lock()
_cache = {}


def _round_f32r(x):
    """Round fp32 -> the f32r-representable grid (13-bit mantissa)."""
    if not USE_F32R:
        return np.ascontiguousarray(x, np.float32)
    u = np.ascontiguousarray(x, np.float32).view(np.uint32)
    return ((u + np.uint32(0x1000)) & np.uint32(0xFFFFE000)).view(np.float32)


# ----------------------------------------------------------------- builders
def _build_stats_nc():
    """Per-core partial sum / sum-of-squares of node_feats over rows.

    Input  nft  [FPAD, RPAD]  (features on partitions, zero-padded)
    Output stats [FPAD, 2]    (col 0 = sum, col 1 = sumsq)
    """
    nc = bacc.Bacc("TRN2", target_bir_lowering=False, debug=False,
                   num_devices=N_CORES)
    nft = nc.dram_tensor("nft", [FPAD, RPAD], FP32, kind="ExternalInput").ap()
    stats = nc.dram_tensor("stats", [FPAD, 2], FP32, kind="ExternalOutput").ap()
    with tile.TileContext(nc) as tc:
        with (
            tc.tile_pool(name="io", bufs=2) as io,
            tc.tile_pool(name="scratch", bufs=2) as sp,
            tc.tile_pool(name="acc", bufs=10) as ap,
        ):
            for k in range(FPAD // 128):
                t = io.tile([128, RPAD], FP32)
                nc.sync.dma_start(t[:], nft[128 * k:128 * (k + 1), :])
                sums = ap.tile([128, 1], FP32, tag="sums")
                nc.vector.reduce_sum(sums[:], t[:], axis=mybir.AxisListType.X)
                sq = sp.tile([128, RPAD], FP32)
                sqs = ap.tile([128, 1], FP32, tag="sqs")
                nc.scalar.activation(sq[:], t[:], AF.Square, accum_out=sqs[:])
                nc.sync.dma_start(stats[128 * k:128 * (k + 1), 0:1], sums[:])
                nc.sync.dma_start(stats[128 * k:128 * (k + 1), 1:2], sqs[:])
    nc.compile()
    return nc


def _build_main_nc(reps=1):
    nc = bacc.Bacc("TRN2", target_bir_lowering=False, debug=False,
                   num_devices=N_CORES)
    dt = nc.dram_tensor
    aps = {}

    def di(name, shape, dtype=MMDT):
        aps[name] = dt(name, shape, dtype, kind="ExternalInput").ap()

    di("img", [128, NPX])
    di("x0r", [NB, NBLK * FPAD])
    di("atbi", [NB, NBLK * ATW])
    di("w1x", [FPAD, 512]); di("w1a", [FPAD, 512])
    di("w2x", [512, 256]); di("w2a", [512, 256])
    di("w3x", [256, 128]); di("w3a", [256, 128])
    di("w4x", [128, 64]); di("w4a", [128, 64])
    di("b1", [1, 512]); di("b2", [1, 256]); di("b3", [1, 128])
    di("b4", [64, 1], FP32)
    di("cw1", [64, 32], FP32); di("cb1", [32, 1], FP32)
    di("pa", [32, 1], FP32)
    di("cw2", [32, 2], FP32); di("cb2", [2, 1], FP32)
    di("cwbd", [128, 32]); di("cb32", [32, 1], FP32)
    di("ones", [1, NB])
    aps["pred"] = dt("pred", [B, CO, HS, W], FP32, kind="ExternalOutput").ap()
    aps["predt"] = dt("predt", [2, RPAD], FP32, kind="ExternalOutput").ap()

    with tile.TileContext(nc) as tc:
        _emit_main(nc, tc, aps, reps)
    nc.compile()
    return nc


def _emit_main(nc, tc, t, reps):
    from contextlib import ExitStack
    ctx = ExitStack()
    with ctx:
        def pool(name, bufs, space="SBUF"):
            return ctx.enter_context(
                tc.tile_pool(name=name, bufs=bufs, space=space))

        consts = pool("consts", 1)
        imgp = pool("imgp", 3)
        cop = pool("cop", 2)
        x0p = pool("x0p", 2)
        atp = pool("atp", 2)
        blkp = pool("blk", 2)
        c4ap = pool("c4ap", 1)
        x4p = pool("x4p", 1)
        predp = pool("predp", 1)
        hp = pool("hp", 2)
        psb = pool("psb", 4, space="PSUM")    # shared psum, slots [128,1024]

        # ---- resident constants / weights -------------------------------
        def load_const(ap_in, shape, tag, dtype=MMDT):
            s = consts.tile(shape, dtype, tag=tag)
            nc.sync.dma_start(s[:], ap_in)
            return s

        def load_chunks(name, nchunk, width):
            return [load_const(t[name][128 * k:128 * (k + 1), :],
                               [128, width], f"{name}{k}")
                    for k in range(nchunk)]

        w1x_s = load_chunks("w1x", 5, 512)
        w1a_s = load_chunks("w1a", 5, 512)
        w2x_s = load_chunks("w2x", 4, 256)
        w2a_s = load_chunks("w2a", 4, 256)
        w3x_s = load_chunks("w3x", 2, 128)
        w3a_s = load_chunks("w3a", 2, 128)
        w4x_s = load_const(t["w4x"][:], [128, 64], "w4x")
        w4a_s = load_const(t["w4a"][:], [128, 64], "w4a")
        b1_s = load_const(t["b1"][:], [1, 512], "b1")
        b2_s = load_const(t["b2"][:], [1, 256], "b2")
        b3_s = load_const(t["b3"][:], [1, 128], "b3")
        b4_s = load_const(t["b4"][:], [64, 1], "b4", FP32)
        cw1_s = load_const(t["cw1"][:], [64, 32], "cw1", FP32)
        cb1_s = load_const(t["cb1"][:], [32, 1], "cb1", FP32)
        pa_s = load_const(t["pa"][:], [32, 1], "pa", FP32)
        cw2_s = load_const(t["cw2"][:], [32, 2], "cw2", FP32)
        cb2_s = load_const(t["cb2"][:], [2, 1], "cb2", FP32)
        cwbd_s = load_const(t["cwbd"][:], [128, 32], "cwbd")
        cb32_s = load_const(t["cb32"][:], [32, 1], "cb32", FP32)
        ones_s = load_const(t["ones"][:], [1, NB], "ones")

        # dest view for conv DMA: h split as (sr, j)
        pred_v = t["pred"].rearrange("b o (S j) w -> S (b o) j w", j=4)

        def body(_iv=None):
            # ---- conv over this core's image slab -----------------------
            # super-round sr covers h-rows 4sr..4sr+3 (chunks j=0..3);
            # f32r matmuls must write PSUM partition base 0, so chunks go
            # side-by-side in the free dim (2 per 2-bank psum tile).
            for sr in range(16):
                if sr % 2 == 0:
                    it = imgp.tile([128, 4096], MMDT, tag="img")
                    nc.sync.dma_start(
                        it[:], t["img"][:, 4096 * (sr // 2):
                                        4096 * (sr // 2 + 1)])
                so = 2048 * (sr % 2)
                out_sb = cop.tile([32, 2048], FP32, tag="convout")
                for half in range(2):
                    ps = psb.tile([32, 1024], FP32, tag="ps")
                    for j2 in range(2):
                        j = 2 * half + j2
                        nc.tensor.matmul(
                            ps[:, 512 * j2:512 * (j2 + 1)],
                            cwbd_s[:, 0:32],
                            it[:, so + 512 * j:so + 512 * (j + 1)],
                            start=True, stop=True)
                    if half == 0:
                        nc.vector.tensor_scalar_add(
                            out_sb[:, 0:1024], ps[:], cb32_s[:])
                    else:
                        nc.scalar.activation(
                            out_sb[:, 1024:2048], ps[:], AF.Identity,
                            bias=cb32_s[:])
                osv = out_sb.rearrange("p (j w) -> p j w", j=4)
                nc.sync.dma_start(pred_v[sr], osv[0:24])

            # ---- GCN block chain ---------------------------------------
            c4a_all = c4ap.tile([128, RPAD], MMDT, tag="c4a")
            c4x_all = c4ap.tile([128, RPAD], MMDT, tag="c4x")
            x4t_all = x4p.tile([64, RPAD], FP32, tag="x4t")
            for b in range(NBLK):
                x0_b = x0p.tile([NB, FPAD], MMDT, tag="x0")
                nc.sync.dma_start(
                    x0_b[:], t["x0r"][:, FPAD * b:FPAD * (b + 1)])
                at_b = atp.tile([NB, ATW], MMDT, tag="at")
                nc.sync.dma_start(
                    at_b[:], t["atbi"][:, ATW * b:ATW * (b + 1)])

                # bmm1: [agg1T | x0T] chunks [*, 256] at free offset 256*fc
                ps_c1a = psb.tile([128, 1024], FP32, tag="ps")
                ps_c1b = psb.tile([128, 1024], FP32, tag="ps")
                c1 = blkp.tile([128, 1280], MMDT, tag="c1")
                for fc in range(5):
                    pst = ps_c1a if fc < 2 else ps_c1b
                    po = 256 * fc if fc < 2 else 256 * (fc - 2)
                    nc.tensor.matmul(
                        pst[:, po:po + 256],
                        x0_b[:, 128 * fc:128 * (fc + 1)],
                        at_b[:], start=True, stop=True)
                nc.vector.tensor_copy(c1[:, 0:512], ps_c1a[:, 0:512])
                nc.vector.tensor_copy(c1[:, 512:1280], ps_c1b[:, 0:768])

                # dense1: x1 = relu([x0,agg1] @ W1' + b1')
                ps_x1 = psb.tile([NB, 512], FP32, tag="ps")
                for k in range(5):
                    nc.tensor.matmul(
                        ps_x1[:], c1[:, 256 * k + NB:256 * k + 2 * NB],
                        w1x_s[k][:], start=(k == 0), stop=False)
                for k in range(5):
                    nc.tensor.matmul(
                        ps_x1[:], c1[:, 256 * k:256 * k + NB],
                        w1a_s[k][:], start=False, stop=False)
                nc.tensor.matmul(ps_x1[:], ones_s[:], b1_s[:],
                                 start=False, stop=True)
                x1 = blkp.tile([NB, 512], MMDT, tag="x1")
                nc.scalar.activation(x1[:], ps_x1[:], AF.Relu)

                # bmm2 combined: [agg2T | x1T] chunks at 256*fc
                ps_c2 = psb.tile([128, 1024], FP32, tag="ps")
                for fc in range(4):
                    nc.tensor.matmul(
                        ps_c2[:, 256 * fc:256 * (fc + 1)],
                        x1[:, 128 * fc:128 * (fc + 1)],
                        at_b[:], start=True, stop=True)
                c2 = blkp.tile([128, 1024], MMDT, tag="c2")
                nc.scalar.copy(c2[:], ps_c2[:])

                ps_x2 = psb.tile([NB, 256], FP32, tag="ps")
                for k in range(4):
                    nc.tensor.matmul(
                        ps_x2[:], c2[:, 256 * k + NB:256 * k + 2 * NB],
                        w2x_s[k][:], start=(k == 0), stop=False)
                for k in range(4):
                    nc.tensor.matmul(
                        ps_x2[:], c2[:, 256 * k:256 * k + NB],
                        w2a_s[k][:], start=False, stop=False)
                nc.tensor.matmul(ps_x2[:], ones_s[:], b2_s[:],
                                 start=False, stop=True)
                x2 = blkp.tile([NB, 256], MMDT, tag="x2")
                nc.scalar.activation(x2[:], ps_x2[:], AF.Relu)

                # bmm3
                ps_c3 = psb.tile([128, 512], FP32, tag="ps")
                for fc in range(2):
                    nc.tensor.matmul(
                        ps_c3[:, 256 * fc:256 * (fc + 1)],
                        x2[:, 128 * fc:128 * (fc + 1)],
                        at_b[:], start=True, stop=True)
                c3 = blkp.tile([128, 512], MMDT, tag="c3")
                nc.vector.tensor_copy(c3[:], ps_c3[:])

                ps_x3 = psb.tile([NB, 128], FP32, tag="ps")
                for k in range(2):
                    nc.tensor.matmul(
                        ps_x3[:], c3[:, 256 * k + NB:256 * k + 2 * NB],
                        w3x_s[k][:], start=(k == 0), stop=False)
                for k in range(2):
                    nc.tensor.matmul(
                        ps_x3[:], c3[:, 256 * k:256 * k + NB],
                        w3a_s[k][:], start=False, stop=False)
                nc.tensor.matmul(ps_x3[:], ones_s[:], b3_s[:],
                                 start=False, stop=True)
                x3 = blkp.tile([NB, 128], MMDT, tag="x3")
                nc.scalar.activation(x3[:], ps_x3[:], AF.Relu)

                # bmm4 -> c4_all (consumed by the batched dense4)
                ps_c4 = psb.tile([128, ATW], FP32, tag="ps")
                nc.tensor.matmul(ps_c4[:], x3[:], at_b[:],
                                 start=True, stop=True)
                nc.vector.tensor_copy(
                    c4a_all[:, NB * b:NB * (b + 1)], ps_c4[:, 0:NB])
                nc.vector.tensor_copy(
                    c4x_all[:, NB * b:NB * (b + 1)], ps_c4[:, NB:2 * NB])

            # dense4 (option B), 4 blocks per matmul: x4T = relu(W4.T catT+b4)
            b0 = 0
            for nb in DGRP:
                ps_x4 = psb.tile([64, 4 * NB], FP32, tag="ps")
                wv = nb * NB
                nc.tensor.matmul(ps_x4[:, 0:wv],
                                 w4x_s[:], c4x_all[:, NB * b0:NB * b0 + wv],
                                 start=True, stop=False)
                nc.tensor.matmul(ps_x4[:, 0:wv],
                                 w4a_s[:], c4a_all[:, NB * b0:NB * b0 + wv],
                                 start=False, stop=True)
                nc.scalar.activation(
                    x4t_all[:, NB * b0:NB * b0 + wv], ps_x4[:, 0:wv],
                    AF.Relu, bias=b4_s[:])
                b0 += nb

            # ---- classifier tail on x4T --------------------------------
            predt_sb = predp.tile([2, RPAD], FP32, tag="predt")
            CH = [512] * 5 + [RPAD - 5 * 512]
            off = 0
            for w in CH:
                ps_h = psb.tile([32, 512], FP32, tag="ps")
                nc.tensor.matmul(ps_h[:, 0:w], cw1_s[:],
                                 x4t_all[:, off:off + w],
                                 start=True, stop=True)
                # PReLU(z) = max(z,0) + a*min(z,0), z = W1h@x4 + b1h
                z_sb = hp.tile([32, 512], FP32, tag="z")
                nc.scalar.activation(z_sb[:, 0:w], ps_h[:, 0:w], AF.Identity,
                                     bias=cb1_s[:])
                hneg = hp.tile([32, 512], FP32, tag="hneg")
                nc.vector.tensor_scalar(hneg[:, 0:w], z_sb[:, 0:w], 0.0,
                                        pa_s[:], mybir.AluOpType.min,
                                        mybir.AluOpType.mult)
                h_sb = hp.tile([32, 512], FP32, tag="h")
                nc.vector.tensor_scalar_max(h_sb[:, 0:w], z_sb[:, 0:w], 0.0)
                nc.vector.tensor_add(h_sb[:, 0:w], h_sb[:, 0:w],
                                     hneg[:, 0:w])
                ps_p = psb.tile([2, 512], FP32, tag="ps")
                nc.tensor.matmul(ps_p[:, 0:w], cw2_s[:], h_sb[:, 0:w],
                                 start=True, stop=True)
                nc.scalar.activation(predt_sb[:, off:off + w], ps_p[:, 0:w],
                                     AF.Identity, bias=cb2_s[:])
                off += w
            nc.sync.dma_start(t["predt"][:], predt_sb[:])

        if reps == 1:
            body()
        else:
            with tc.For_i(0, reps, 1) as iv:
                body(iv)


# ------------------------------------------------------------- host prep
def _pad_rows(sh):
    """[GC*N, F] -> [RPAD, F] with zero pad rows per 3-graph block."""
    rp = np.zeros((RPAD, F), np.float32)
    full = (GC * N) // (GPB * N)          # 21 full blocks
    rp[:full * NB] = sh[:full * GPB * N].reshape(full * NB, F)
    rem = GC * N - full * GPB * N
    if rem:
        rp[full * NB:full * NB + rem] = sh[full * GPB * N:]
    return rp


def _prep_stats_inputs(node_feats):
    nf = np.ascontiguousarray(node_feats, np.float32).reshape(G, N, F)
    maps = []
    for c in range(N_CORES):
        rp = _pad_rows(nf[c * GC:(c + 1) * GC].reshape(GC * N, F))
        nft = np.zeros((FPAD, RPAD), np.float32)
        nft[:F, :] = rp.T
        maps.append({"nft": nft})
    return maps


def _prep_main_inputs(inputs, node_feats, A, conv_w, conv_b, w1s, b1s,
                      W2, b2v, W3, b3v, W4, b4v,
                      cls_w1, cls_b1, prelu_a, cls_w2, cls_b2):
    f32 = lambda x: np.ascontiguousarray(x, np.float32)
    R = _round_f32r
    inputs, node_feats, A = f32(inputs), f32(node_feats), f32(A)
    cwbd = np.zeros((128, 32), np.float32)
    cb32 = np.zeros((32, 1), np.float32)
    wt = f32(conv_w).T  # [32, 6]
    for b in range(B):
        cwbd[32 * b:32 * (b + 1), 6 * b:6 * (b + 1)] = wt
        cb32[6 * b:6 * (b + 1), 0] = f32(conv_b)
    w1xp = np.zeros((FPAD, 512), np.float32)
    w1ap = np.zeros((FPAD, 512), np.float32)
    w1xp[:F] = w1s[:F]
    w1ap[:F] = w1s[F:]
    shared = {
        "w1x": R(w1xp), "w1a": R(w1ap),
        "w2x": R(W2[:512]), "w2a": R(W2[512:]),
        "w3x": R(W3[:256]), "w3a": R(W3[256:]),
        "w4x": R(W4[:128]), "w4a": R(W4[128:]),
        "b1": R(f32(b1s).reshape(1, 512)), "b2": R(f32(b2v).reshape(1, 256)),
        "b3": R(f32(b3v).reshape(1, 128)), "b4": f32(b4v).reshape(64, 1),
        "cw1": f32(cls_w1), "cb1": f32(cls_b1).reshape(32, 1),
        "pa": f32(prelu_a).reshape(32, 1), "cw2": f32(cls_w2),
        "cb2": f32(cls_b2).reshape(2, 1),
        "cwbd": R(cwbd), "cb32": cb32,
        "ones": np.ones((1, NB), np.float32),
    }
    # atbi: [123, NBLK*256]: per block [blockdiag(A_g^T) | I | 0]
    At = A.transpose(0, 2, 1)             # [G, 41, 41] = A^T per graph
    eye = np.eye(NB, dtype=np.float32)
    maps = []
    for c in range(N_CORES):
        img = inputs[:, :, c * HS:(c + 1) * HS, :].reshape(128, NPX)
        rp = _pad_rows(node_feats[c * GC:(c + 1) * GC].reshape(GC * N, F))
        x0r = np.zeros((NB, NBLK, FPAD), np.float32)
        x0r[:, :, :F] = rp.reshape(NBLK, NB, F).transpose(1, 0, 2)
        atbi = np.zeros((NB, NBLK, ATW), np.float32)
        Ash = At[c * GC:(c + 1) * GC]
        for b in range(NBLK):
            ng = min(GPB, GC - b * GPB)
            for gi in range(ng):
                atbi[N * gi:N * (gi + 1), b, N * gi:N * (gi + 1)] = \
                    Ash[b * GPB + gi]
            atbi[:, b, NB:2 * NB] = eye
        m = dict(shared)
        m.update({"img": R(img), "x0r": R(x0r.reshape(NB, NBLK * FPAD)),
                  "atbi": R(atbi.reshape(NB, NBLK * ATW))})
        maps.append(m)
    return maps


# ------------------------------------------------------------- execution
def _output_names(nc):
    names = []
    for alloc in nc.m.functions[0].allocations:
        if isinstance(alloc, mybir.MemoryLocationSet) and \
                alloc.kind == "ExternalOutput":
            names.append(alloc.memorylocations[0].name)
    return names


def _run(nc, in_maps):
    if os.environ.get("KERNEL_SIM"):
        from concourse.bass_interp import MultiCoreSim
        sim = MultiCoreSim(nc, num_cores=N_CORES, trace=False)
        for c in range(N_CORES):
            for k, v in in_maps[c].items():
                sim.cores[c].tensor(k)[:] = v
        sim.simulate()
        onames = _output_names(nc)
        return [{n: np.array(sim.cores[c].tensor(n)) for n in onames}
                for c in range(N_CORES)]
    from concourse.bass_utils import run_bass_kernel_spmd
    return run_bass_kernel_spmd(nc, in_maps, list(range(N_CORES))).results


def _get_nc(which, reps=1):
    key = (which, reps)
    with _lock:
        if key not in _cache:
            if which == "stats":
                _cache[key] = _build_stats_nc()
            else:
                _cache[key] = _build_main_nc(reps)
        return _cache[key]


def kernel(inputs, node_feats, A, knn_inx, conv_w, conv_b, W1, b1, W2, b2,
           W3, b3, W4, b4, cls_w1, cls_b1, prelu_a, cls_w2, cls_b2):
    f32 = lambda x: np.asarray(x, np.float32)
    # ---- dispatch A: BN statistics ------------------------------------
    nc_s = _get_nc("stats")
    smaps = _prep_stats_inputs(node_feats)
    sres = _run(nc_s, smaps)
    tot = np.zeros((FPAD, 2), np.float64)
    for c in range(N_CORES):
        tot += sres[c]["stats"].astype(np.float64)
    cnt = float(G * N)
    mean = (tot[:F, 0] / cnt).astype(np.float32)
    var = (tot[:F, 1] / cnt).astype(np.float32) - mean * mean
    s = (1.0 / np.sqrt(var + np.float32(EPS))).astype(np.float32)
    t_ = (-mean * s).astype(np.float32)
    W1 = f32(W1)
    sdup = np.concatenate([s, s]).astype(np.float32)
    w1s = W1 * sdup[:, None]
    b1s = f32(b1) + t_ @ W1[:F] + t_ @ W1[F:]

    # ---- dispatch B: conv + GCN ---------------------------------------
    nc_m = _get_nc("main")
    mmaps = _prep_main_inputs(inputs, node_feats, A, conv_w, conv_b,
                              w1s, b1s, W2, b2, W3, b3, W4, b4,
                              cls_w1, cls_b1, prelu_a, cls_w2, cls_b2)
    mres = _run(nc_m, mmaps)

    # ---- host gather/unshard ------------------------------------------
    pred_maps = np.concatenate(
        [mres[c]["pred"] for c in range(N_CORES)], axis=2)
    x4pred = np.zeros((G * N, 2), np.float32)
    for c in range(N_CORES):
        pt = mres[c]["predt"]  # [2, RPAD]
        sh = pt.T.reshape(NBLK, NB, 2)
        full = (GC * N) // (GPB * N)
        rem = GC * N - full * GPB * N
        rows = [sh[:full].reshape(full * NB, 2)]
        if rem:
            rows.append(sh[full, :rem])
        x4pred[c * GC * N:(c + 1) * GC * N] = np.concatenate(rows, 0)
    x4pred = x4pred.reshape(G, N, 2)
    ki = np.asarray(knn_inx)
    gcn_pred = np.take_along_axis(
        x4pred, ki[:, :, None].astype(np.int64), axis=1).reshape(-1, 2)
    return pred_maps, gcn_pred


# revision 13
# speedup vs baseline: 296.0737x; 1.2954x over previous

# Trainium2 Bass kernel for nn_DRRGHead (1x1 conv head + 4-layer GraphConv GCN
# over 512 independent local graphs + knn-gather classifier tail).
#
# Sharding (8 cores, data-parallel):
#   - image: H=512 split into 8 slabs of 64 rows; each core convolves
#     (4 batches x 32 ch x 64 rows x 512 cols) with the 6x32 1x1-conv.
#   - graphs: G=512 split into 64 graphs/core, processed as 22 blocks of
#     3 graphs (3*41=123 rows on partitions; last block 1 graph, zero-padded).
#
# BatchNorm handling: BN(x) = x*s + t is affine and A is row-normalized
# (A @ 1 == 1), so BN folds into layer-1 weights/bias:
#   [BN(x), A BN(x)] @ W1 + b1 == [x, A x] @ (s_dup*W1) + (b1 + t@(W1a+W1b)).
# A small first dispatch computes per-core partial sums/sumsq of node_feats
# (the full 48-MB reduction stays on device); the host only adds the eight
# partial [640,2] vectors, forms s,t and rescales W1 (0.3 MFLOP of glue).
#
# Layout scheme on device (per block of 123 rows):
#   x_l kept rows-on-partitions ("L_R").  One PE matmul per feature-chunk
#   with rhs = [blockdiag(A_g^T) | I_123 | 0] (256 wide) yields
#   [agg_l^T | x_l^T] (features-on-partitions, "L_F") directly -- the
#   transpose needed by the next dense layer falls out of the same matmul,
#   so the whole GCN chain needs zero explicit transposes.  Dense layers
#   contract features: lhsT = catT chunks (L_F), rhs = W chunks -> x_{l+1}
#   in L_R.  Layer 4 instead uses lhsT = W4 chunks and batches 4 blocks per
#   matmul (N=492) because the tail wants x4^T.  Bias is added with a
#   rank-1 matmul (ones[1,123] x b[1,outs]) into PSUM; ReLU on eviction.
#
# Matmul dtype: float32r (fp32 container, ~13-bit mantissa used by the PE
# at 1 cycle/row for free-dim >= 256, vs 4 cycles/row for plain fp32).
# HW-verified: f32r matmul is exact on pre-rounded inputs; intermediate
# evictions to f32r tiles round once (~1e-4 rel).  End-to-end impact is
# ~2e-4 on gcn_pred; plain-fp32 fallback via KERNEL_FP32=1.
#
# Conv: lhsT = blockdiag over 4 batches of conv_w^T (128x32, cols 24..31
# zero), rhs = pixel chunks [128, 512]; 4 chunks run concurrently in
# distinct PE col-strips via tile_position=(0,32j), evicted 128-lanes-wide
# with per-partition bias.

import os
import sys
import threading

import numpy as np

if "/opt/trn_rl_repo" not in sys.path:
    sys.path.insert(0, "/opt/trn_rl_repo")

import concourse.bass as bass  # noqa: F401
import concourse.tile as tile
from concourse import bacc, mybir

FP32 = mybir.dt.float32
F32R = mybir.dt.float32r
AF = mybir.ActivationFunctionType

USE_F32R = not os.environ.get("KERNEL_FP32")
MMDT = F32R if USE_F32R else FP32

N_CORES = 8
B, C, H, W = 4, 32, 512, 512
G, N, F = 512, 41, 576
CO = 6                     # conv out channels
HS = H // N_CORES          # 64 image rows per core
NPX = HS * W               # 32768 pixels per (core, batch)
GC = G // N_CORES          # 64 graphs per core
GPB = 3                    # graphs per block
NB = 123                   # rows per block (3*41)
NBLK = 22                  # blocks per core (21 full + 1 single-graph)
RPAD = NBLK * NB           # 2706 padded rows per core
ATW = 256                  # per-block [blockdiag(A^T) | I | 0pad] width
FPAD = 640                 # 576 padded to 5*128
EPS = 1e-5
DGRP = [4, 4, 4, 4, 4, 2]  # dense-4 block batching

_lock = threading.Lock()
_cache = {}


def _round_f32r(x):
    """Round fp32 -> the f32r-representable grid (13-bit mantissa)."""
    if not USE_F32R:
        return np.ascontiguousarray(x, np.float32)
    u = np.ascontiguousarray(x, np.float32).view(np.uint32)
    return ((u + np.uint32(0x1000)) & np.uint32(0xFFFFE000)).view(np.float32)


# ----------------------------------------------------------------- builders
def _build_stats_nc():
    """Per-core partial sum / sum-of-squares of node_feats over rows.

    Input  nft  [FPAD, RPAD]  (features on partitions, zero-padded)
    Output stats [FPAD, 2]    (col 0 = sum, col 1 = sumsq)
    """
    nc = bacc.Bacc("TRN2", target_bir_lowering=False, debug=False,
                   num_devices=N_CORES)
    nft = nc.dram_tensor("nft", [FPAD, RPAD], FP32, kind="ExternalInput").ap()
    stats = nc.dram_tensor("stats", [FPAD, 2], FP32, kind="ExternalOutput").ap()
    with tile.TileContext(nc) as tc:
        with (
            tc.tile_pool(name="io", bufs=2) as io,
            tc.tile_pool(name="scratch", bufs=2) as sp,
            tc.tile_pool(name="acc", bufs=10) as ap,
        ):
            for k in range(FPAD // 128):
                t = io.tile([128, RPAD], FP32)
                nc.sync.dma_start(t[:], nft[128 * k:128 * (k + 1), :])
                sums = ap.tile([128, 1], FP32, tag="sums")
                nc.vector.reduce_sum(sums[:], t[:], axis=mybir.AxisListType.X)
                sq = sp.tile([128, RPAD], FP32)
                sqs = ap.tile([128, 1], FP32, tag="sqs")
                nc.scalar.activation(sq[:], t[:], AF.Square, accum_out=sqs[:])
                nc.sync.dma_start(stats[128 * k:128 * (k + 1), 0:1], sums[:])
                nc.sync.dma_start(stats[128 * k:128 * (k + 1), 1:2], sqs[:])
    nc.compile()
    return nc


def _build_main_nc(reps=1):
    nc = bacc.Bacc("TRN2", target_bir_lowering=False, debug=False,
                   num_devices=N_CORES)
    dt = nc.dram_tensor
    aps = {}

    def di(name, shape, dtype=MMDT):
        aps[name] = dt(name, shape, dtype, kind="ExternalInput").ap()

    di("img", [128, NPX])
    di("x0r", [NB, NBLK * FPAD])
    di("atbi", [NB, NBLK * ATW])
    di("w1x", [FPAD, 512]); di("w1a", [FPAD, 512])
    di("w2x", [512, 256]); di("w2a", [512, 256])
    di("w3x", [256, 128]); di("w3a", [256, 128])
    di("w4x", [128, 64]); di("w4a", [128, 64])
    di("b1", [1, 512]); di("b2", [1, 256]); di("b3", [1, 128])
    di("b4", [64, 1], FP32)
    di("cw1", [64, 32], FP32); di("cb1", [32, 1], FP32)
    di("pa", [32, 1], FP32)
    di("cw2", [32, 2], FP32); di("cb2", [2, 1], FP32)
    di("cwbd", [128, 32]); di("cb32", [32, 1], FP32)
    di("ones", [1, NB])
    aps["pred"] = dt("pred", [B, CO, HS, W], FP32, kind="ExternalOutput").ap()
    aps["predt"] = dt("predt", [2, RPAD], FP32, kind="ExternalOutput").ap()

    with tile.TileContext(nc) as tc:
        _emit_main(nc, tc, aps, reps)
    nc.compile()
    return nc


def _emit_main(nc, tc, t, reps):
    from contextlib import ExitStack
    ctx = ExitStack()
    with ctx:
        def pool(name, bufs, space="SBUF"):
            return ctx.enter_context(
                tc.tile_pool(name=name, bufs=bufs, space=space))

        consts = pool("consts", 1)
        imgp = pool("imgp", 3)
        cop = pool("cop", 2)
        x0p = pool("x0p", 3)
        atp = pool("atp", 3)
        blkp = pool("blk", 3)
        c4ap = pool("c4ap", 1)
        x4p = pool("x4p", 1)
        predp = pool("predp", 1)
        hp = pool("hp", 2)
        psb = pool("psb", 4, space="PSUM")    # shared psum, slots [128,1024]

        # ---- resident constants / weights -------------------------------
        def load_const(ap_in, shape, tag, dtype=MMDT):
            s = consts.tile(shape, dtype, tag=tag)
            nc.sync.dma_start(s[:], ap_in)
            return s

        def load_chunks(name, nchunk, width):
            return [load_const(t[name][128 * k:128 * (k + 1), :],
                               [128, width], f"{name}{k}")
                    for k in range(nchunk)]

        w1x_s = load_chunks("w1x", 5, 512)
        w1a_s = load_chunks("w1a", 5, 512)
        w2x_s = load_chunks("w2x", 4, 256)
        w2a_s = load_chunks("w2a", 4, 256)
        w3x_s = load_chunks("w3x", 2, 128)
        w3a_s = load_chunks("w3a", 2, 128)
        w4x_s = load_const(t["w4x"][:], [128, 64], "w4x")
        w4a_s = load_const(t["w4a"][:], [128, 64], "w4a")
        b1_s = load_const(t["b1"][:], [1, 512], "b1")
        b2_s = load_const(t["b2"][:], [1, 256], "b2")
        b3_s = load_const(t["b3"][:], [1, 128], "b3")
        b4_s = load_const(t["b4"][:], [64, 1], "b4", FP32)
        cw1_s = load_const(t["cw1"][:], [64, 32], "cw1", FP32)
        cb1_s = load_const(t["cb1"][:], [32, 1], "cb1", FP32)
        pa_s = load_const(t["pa"][:], [32, 1], "pa", FP32)
        cw2_s = load_const(t["cw2"][:], [32, 2], "cw2", FP32)
        cb2_s = load_const(t["cb2"][:], [2, 1], "cb2", FP32)
        cwbd_s = load_const(t["cwbd"][:], [128, 32], "cwbd")
        cb32_s = load_const(t["cb32"][:], [32, 1], "cb32", FP32)
        ones_s = load_const(t["ones"][:], [1, NB], "ones")

        # dest view for conv DMA: h split as (sr, j)
        pred_v = t["pred"].rearrange("b o (S j) w -> S (b o) j w", j=4)

        def body(_iv=None):
            # ---- conv over this core's image slab -----------------------
            # super-round sr covers h-rows 4sr..4sr+3 (chunks j=0..3);
            # f32r matmuls must write PSUM partition base 0, so chunks go
            # side-by-side in the free dim (2 per 2-bank psum tile).
            for sr in range(16):
                it = imgp.tile([128, 2048], MMDT, tag="img")
                nc.sync.dma_start(
                    it[:], t["img"][:, 2048 * sr:2048 * (sr + 1)])
                so = 0
                out_sb = cop.tile([32, 2048], FP32, tag="convout")
                for half in range(2):
                    ps = psb.tile([32, 1024], FP32, tag="ps")
                    for j2 in range(2):
                        j = 2 * half + j2
                        nc.tensor.matmul(
                            ps[:, 512 * j2:512 * (j2 + 1)],
                            cwbd_s[:, 0:32],
                            it[:, so + 512 * j:so + 512 * (j + 1)],
                            start=True, stop=True)
                    if half == 0:
                        nc.vector.tensor_scalar_add(
                            out_sb[:, 0:1024], ps[:], cb32_s[:])
                    else:
                        nc.scalar.activation(
                            out_sb[:, 1024:2048], ps[:], AF.Identity,
                            bias=cb32_s[:])
                osv = out_sb.rearrange("p (j w) -> p j w", j=4)
                nc.sync.dma_start(pred_v[sr], osv[0:24])

            # ---- GCN block chain ---------------------------------------
            c4a_all = c4ap.tile([128, RPAD], MMDT, tag="c4a")
            c4x_all = c4ap.tile([128, RPAD], MMDT, tag="c4x")
            x4t_all = x4p.tile([64, RPAD], FP32, tag="x4t")
            def stage_load(b):
                x0_b = x0p.tile([NB, FPAD], MMDT, tag="x0")
                nc.sync.dma_start(
                    x0_b[:], t["x0r"][:, FPAD * b:FPAD * (b + 1)])
                at_b = atp.tile([NB, ATW], MMDT, tag="at")
                nc.sync.dma_start(
                    at_b[:], t["atbi"][:, ATW * b:ATW * (b + 1)])
                return x0_b, at_b

            def stage_bmm1(b, st):
                x0_b, at_b = st["ld"]
                ps_c1a = psb.tile([128, 1024], FP32, tag="ps")
                ps_c1b = psb.tile([128, 1024], FP32, tag="ps")
                c1 = blkp.tile([128, 1280], MMDT, tag="c1")
                for fc in range(5):
                    pst = ps_c1a if fc < 2 else ps_c1b
                    po = 256 * fc if fc < 2 else 256 * (fc - 2)
                    nc.tensor.matmul(
                        pst[:, po:po + 256],
                        x0_b[:, 128 * fc:128 * (fc + 1)],
                        at_b[:], start=True, stop=True)
                nc.vector.tensor_copy(c1[:, 0:512], ps_c1a[:, 0:512])
                nc.vector.tensor_copy(c1[:, 512:1280], ps_c1b[:, 0:768])
                st["c1"] = c1

            def stage_dense1(b, st):
                c1 = st["c1"]
                ps_x1 = psb.tile([NB, 512], FP32, tag="ps")
                for k in range(5):
                    nc.tensor.matmul(
                        ps_x1[:], c1[:, 256 * k + NB:256 * k + 2 * NB],
                        w1x_s[k][:], start=(k == 0), stop=False)
                for k in range(5):
                    nc.tensor.matmul(
                        ps_x1[:], c1[:, 256 * k:256 * k + NB],
                        w1a_s[k][:], start=False, stop=False)
                nc.tensor.matmul(ps_x1[:], ones_s[:], b1_s[:],
                                 start=False, stop=True)
                x1 = blkp.tile([NB, 512], MMDT, tag="x1")
                nc.scalar.activation(x1[:], ps_x1[:], AF.Relu)
                st["x1"] = x1

            def stage_bmm2(b, st):
                at_b = st["ld"][1]
                x1 = st["x1"]
                ps_c2 = psb.tile([128, 1024], FP32, tag="ps")
                for fc in range(4):
                    nc.tensor.matmul(
                        ps_c2[:, 256 * fc:256 * (fc + 1)],
                        x1[:, 128 * fc:128 * (fc + 1)],
                        at_b[:], start=True, stop=True)
                c2 = blkp.tile([128, 1024], MMDT, tag="c2")
                nc.scalar.copy(c2[:], ps_c2[:])
                st["c2"] = c2

            def stage_dense2(b, st):
                c2 = st["c2"]
                ps_x2 = psb.tile([NB, 256], FP32, tag="ps")
                for k in range(4):
                    nc.tensor.matmul(
                        ps_x2[:], c2[:, 256 * k + NB:256 * k + 2 * NB],
                        w2x_s[k][:], start=(k == 0), stop=False)
                for k in range(4):
                    nc.tensor.matmul(
                        ps_x2[:], c2[:, 256 * k:256 * k + NB],
                        w2a_s[k][:], start=False, stop=False)
                nc.tensor.matmul(ps_x2[:], ones_s[:], b2_s[:],
                                 start=False, stop=True)
                x2 = blkp.tile([NB, 256], MMDT, tag="x2")
                nc.scalar.activation(x2[:], ps_x2[:], AF.Relu)
                st["x2"] = x2

            def stage_bmm3(b, st):
                at_b = st["ld"][1]
                x2 = st["x2"]
                ps_c3 = psb.tile([128, 512], FP32, tag="ps")
                for fc in range(2):
                    nc.tensor.matmul(
                        ps_c3[:, 256 * fc:256 * (fc + 1)],
                        x2[:, 128 * fc:128 * (fc + 1)],
                        at_b[:], start=True, stop=True)
                c3 = blkp.tile([128, 512], MMDT, tag="c3")
                nc.vector.tensor_copy(c3[:], ps_c3[:])
                st["c3"] = c3

            def stage_dense3(b, st):
                c3 = st["c3"]
                ps_x3 = psb.tile([NB, 128], FP32, tag="ps")
                for k in range(2):
                    nc.tensor.matmul(
                        ps_x3[:], c3[:, 256 * k + NB:256 * k + 2 * NB],
                        w3x_s[k][:], start=(k == 0), stop=False)
                for k in range(2):
                    nc.tensor.matmul(
                        ps_x3[:], c3[:, 256 * k:256 * k + NB],
                        w3a_s[k][:], start=False, stop=False)
                nc.tensor.matmul(ps_x3[:], ones_s[:], b3_s[:],
                                 start=False, stop=True)
                x3 = blkp.tile([NB, 128], MMDT, tag="x3")
                nc.scalar.activation(x3[:], ps_x3[:], AF.Relu)
                st["x3"] = x3

            def stage_bmm4(b, st):
                at_b = st["ld"][1]
                x3 = st["x3"]
                ps_c4 = psb.tile([128, ATW], FP32, tag="ps")
                nc.tensor.matmul(ps_c4[:], x3[:], at_b[:],
                                 start=True, stop=True)
                nc.vector.tensor_copy(
                    c4a_all[:, NB * b:NB * (b + 1)], ps_c4[:, 0:NB])
                nc.vector.tensor_copy(
                    c4x_all[:, NB * b:NB * (b + 1)], ps_c4[:, NB:2 * NB])

            stages = [stage_bmm1, stage_dense1, stage_bmm2, stage_dense2,
                      stage_bmm3, stage_dense3, stage_bmm4]
            # emit pairs of blocks stage-interleaved so one block's PSUM
            # eviction hides behind the other block's matmuls (PE executes
            # its stream in emission order)
            for p in range(NBLK // 2):
                b0, b1 = 2 * p, 2 * p + 1
                s0 = {"ld": stage_load(b0)}
                s1 = {"ld": stage_load(b1)}
                for stg in stages:
                    stg(b0, s0)
                    stg(b1, s1)

            # dense4 (option B), 4 blocks per matmul: x4T = relu(W4.T catT+b4)
            b0 = 0
            for nb in DGRP:
                ps_x4 = psb.tile([64, 4 * NB], FP32, tag="ps")
                wv = nb * NB
                nc.tensor.matmul(ps_x4[:, 0:wv],
                                 w4x_s[:], c4x_all[:, NB * b0:NB * b0 + wv],
                                 start=True, stop=False)
                nc.tensor.matmul(ps_x4[:, 0:wv],
                                 w4a_s[:], c4a_all[:, NB * b0:NB * b0 + wv],
                                 start=False, stop=True)
                nc.scalar.activation(
                    x4t_all[:, NB * b0:NB * b0 + wv], ps_x4[:, 0:wv],
                    AF.Relu, bias=b4_s[:])
                b0 += nb

            # ---- classifier tail on x4T --------------------------------
            predt_sb = predp.tile([2, RPAD], FP32, tag="predt")
            CH = [512] * 5 + [RPAD - 5 * 512]
            off = 0
            for w in CH:
                ps_h = psb.tile([32, 512], FP32, tag="ps")
                nc.tensor.matmul(ps_h[:, 0:w], cw1_s[:],
                                 x4t_all[:, off:off + w],
                                 start=True, stop=True)
                # PReLU(z) = max(z,0) + a*min(z,0), z = W1h@x4 + b1h
                z_sb = hp.tile([32, 512], FP32, tag="z")
                nc.scalar.activation(z_sb[:, 0:w], ps_h[:, 0:w], AF.Identity,
                                     bias=cb1_s[:])
                hneg = hp.tile([32, 512], FP32, tag="hneg")
                nc.vector.tensor_scalar(hneg[:, 0:w], z_sb[:, 0:w], 0.0,
                                        pa_s[:], mybir.AluOpType.min,
                                        mybir.AluOpType.mult)
                h_sb = hp.tile([32, 512], FP32, tag="h")
                nc.vector.tensor_scalar_max(h_sb[:, 0:w], z_sb[:, 0:w], 0.0)
                nc.vector.tensor_add(h_sb[:, 0:w], h_sb[:, 0:w],
                                     hneg[:, 0:w])
                ps_p = psb.tile([2, 512], FP32, tag="ps")
                nc.tensor.matmul(ps_p[:, 0:w], cw2_s[:], h_sb[:, 0:w],
                                 start=True, stop=True)
                nc.scalar.activation(predt_sb[:, off:off + w], ps_p[:, 0:w],
                                     AF.Identity, bias=cb2_s[:])
                off += w
            nc.sync.dma_start(t["predt"][:], predt_sb[:])

        if reps == 1:
            body()
        else:
            with tc.For_i(0, reps, 1) as iv:
                body(iv)


# ------------------------------------------------------------- host prep
def _pad_rows(sh):
    """[GC*N, F] -> [RPAD, F] with zero pad rows per 3-graph block."""
    rp = np.zeros((RPAD, F), np.float32)
    full = (GC * N) // (GPB * N)          # 21 full blocks
    rp[:full * NB] = sh[:full * GPB * N].reshape(full * NB, F)
    rem = GC * N - full * GPB * N
    if rem:
        rp[full * NB:full * NB + rem] = sh[full * GPB * N:]
    return rp


def _prep_stats_inputs(node_feats):
    nf = np.ascontiguousarray(node_feats, np.float32).reshape(G, N, F)
    maps = []
    for c in range(N_CORES):
        rp = _pad_rows(nf[c * GC:(c + 1) * GC].reshape(GC * N, F))
        nft = np.zeros((FPAD, RPAD), np.float32)
        nft[:F, :] = rp.T
        maps.append({"nft": nft})
    return maps


def _prep_main_inputs(inputs, node_feats, A, conv_w, conv_b, w1s, b1s,
                      W2, b2v, W3, b3v, W4, b4v,
                      cls_w1, cls_b1, prelu_a, cls_w2, cls_b2):
    f32 = lambda x: np.ascontiguousarray(x, np.float32)
    R = _round_f32r
    inputs, node_feats, A = f32(inputs), f32(node_feats), f32(A)
    cwbd = np.zeros((128, 32), np.float32)
    cb32 = np.zeros((32, 1), np.float32)
    wt = f32(conv_w).T  # [32, 6]
    for b in range(B):
        cwbd[32 * b:32 * (b + 1), 6 * b:6 * (b + 1)] = wt
        cb32[6 * b:6 * (b + 1), 0] = f32(conv_b)
    w1xp = np.zeros((FPAD, 512), np.float32)
    w1ap = np.zeros((FPAD, 512), np.float32)
    w1xp[:F] = w1s[:F]
    w1ap[:F] = w1s[F:]
    shared = {
        "w1x": R(w1xp), "w1a": R(w1ap),
        "w2x": R(W2[:512]), "w2a": R(W2[512:]),
        "w3x": R(W3[:256]), "w3a": R(W3[256:]),
        "w4x": R(W4[:128]), "w4a": R(W4[128:]),
        "b1": R(f32(b1s).reshape(1, 512)), "b2": R(f32(b2v).reshape(1, 256)),
        "b3": R(f32(b3v).reshape(1, 128)), "b4": f32(b4v).reshape(64, 1),
        "cw1": f32(cls_w1), "cb1": f32(cls_b1).reshape(32, 1),
        "pa": f32(prelu_a).reshape(32, 1), "cw2": f32(cls_w2),
        "cb2": f32(cls_b2).reshape(2, 1),
        "cwbd": R(cwbd), "cb32": cb32,
        "ones": np.ones((1, NB), np.float32),
    }
    # atbi: [123, NBLK*256]: per block [blockdiag(A_g^T) | I | 0]
    At = A.transpose(0, 2, 1)             # [G, 41, 41] = A^T per graph
    eye = np.eye(NB, dtype=np.float32)
    maps = []
    for c in range(N_CORES):
        img = inputs[:, :, c * HS:(c + 1) * HS, :].reshape(128, NPX)
        rp = _pad_rows(node_feats[c * GC:(c + 1) * GC].reshape(GC * N, F))
        x0r = np.zeros((NB, NBLK, FPAD), np.float32)
        x0r[:, :, :F] = rp.reshape(NBLK, NB, F).transpose(1, 0, 2)
        atbi = np.zeros((NB, NBLK, ATW), np.float32)
        Ash = At[c * GC:(c + 1) * GC]
        for b in range(NBLK):
            ng = min(GPB, GC - b * GPB)
            for gi in range(ng):
                atbi[N * gi:N * (gi + 1), b, N * gi:N * (gi + 1)] = \
                    Ash[b * GPB + gi]
            atbi[:, b, NB:2 * NB] = eye
        m = dict(shared)
        m.update({"img": R(img), "x0r": R(x0r.reshape(NB, NBLK * FPAD)),
                  "atbi": R(atbi.reshape(NB, NBLK * ATW))})
        maps.append(m)
    return maps


# ------------------------------------------------------------- execution
def _output_names(nc):
    names = []
    for alloc in nc.m.functions[0].allocations:
        if isinstance(alloc, mybir.MemoryLocationSet) and \
                alloc.kind == "ExternalOutput":
            names.append(alloc.memorylocations[0].name)
    return names


def _run(nc, in_maps):
    if os.environ.get("KERNEL_SIM"):
        from concourse.bass_interp import MultiCoreSim
        sim = MultiCoreSim(nc, num_cores=N_CORES, trace=False)
        for c in range(N_CORES):
            for k, v in in_maps[c].items():
                sim.cores[c].tensor(k)[:] = v
        sim.simulate()
        onames = _output_names(nc)
        return [{n: np.array(sim.cores[c].tensor(n)) for n in onames}
                for c in range(N_CORES)]
    from concourse.bass_utils import run_bass_kernel_spmd
    return run_bass_kernel_spmd(nc, in_maps, list(range(N_CORES))).results


def _get_nc(which, reps=1):
    key = (which, reps)
    with _lock:
        if key not in _cache:
            if which == "stats":
                _cache[key] = _build_stats_nc()
            else:
                _cache[key] = _build_main_nc(reps)
        return _cache[key]


def kernel(inputs, node_feats, A, knn_inx, conv_w, conv_b, W1, b1, W2, b2,
           W3, b3, W4, b4, cls_w1, cls_b1, prelu_a, cls_w2, cls_b2):
    f32 = lambda x: np.asarray(x, np.float32)
    # ---- dispatch A: BN statistics ------------------------------------
    nc_s = _get_nc("stats")
    smaps = _prep_stats_inputs(node_feats)
    sres = _run(nc_s, smaps)
    tot = np.zeros((FPAD, 2), np.float64)
    for c in range(N_CORES):
        tot += sres[c]["stats"].astype(np.float64)
    cnt = float(G * N)
    mean = (tot[:F, 0] / cnt).astype(np.float32)
    var = (tot[:F, 1] / cnt).astype(np.float32) - mean * mean
    s = (1.0 / np.sqrt(var + np.float32(EPS))).astype(np.float32)
    t_ = (-mean * s).astype(np.float32)
    W1 = f32(W1)
    sdup = np.concatenate([s, s]).astype(np.float32)
    w1s = W1 * sdup[:, None]
    b1s = f32(b1) + t_ @ W1[:F] + t_ @ W1[F:]

    # ---- dispatch B: conv + GCN ---------------------------------------
    nc_m = _get_nc("main")
    mmaps = _prep_main_inputs(inputs, node_feats, A, conv_w, conv_b,
                              w1s, b1s, W2, b2, W3, b3, W4, b4,
                              cls_w1, cls_b1, prelu_a, cls_w2, cls_b2)
    mres = _run(nc_m, mmaps)

    # ---- host gather/unshard ------------------------------------------
    pred_maps = np.concatenate(
        [mres[c]["pred"] for c in range(N_CORES)], axis=2)
    x4pred = np.zeros((G * N, 2), np.float32)
    for c in range(N_CORES):
        pt = mres[c]["predt"]  # [2, RPAD]
        sh = pt.T.reshape(NBLK, NB, 2)
        full = (GC * N) // (GPB * N)
        rem = GC * N - full * GPB * N
        rows = [sh[:full].reshape(full * NB, 2)]
        if rem:
            rows.append(sh[full, :rem])
        x4pred[c * GC * N:(c + 1) * GC * N] = np.concatenate(rows, 0)
    x4pred = x4pred.reshape(G, N, 2)
    ki = np.asarray(knn_inx)
    gcn_pred = np.take_along_axis(
        x4pred, ki[:, :, None].astype(np.int64), axis=1).reshape(-1, 2)
    return pred_maps, gcn_pred


# revision 16
# speedup vs baseline: 352.6354x; 1.1910x over previous

# Trainium2 Bass kernel for nn_DRRGHead (1x1 conv head + 4-layer GraphConv GCN
# over 512 independent local graphs + knn-gather classifier tail).
#
# Sharding (8 cores, data-parallel):
#   - image: H=512 split into 8 slabs of 64 rows; each core convolves
#     (4 batches x 32 ch x 64 rows x 512 cols) with the 6x32 1x1-conv.
#   - graphs: G=512 split into 64 graphs/core, processed as 22 blocks of
#     3 graphs (3*41=123 rows on partitions; last block 1 graph, zero-padded).
#
# BatchNorm handling: BN(x) = x*s + t is affine and A is row-normalized
# (A @ 1 == 1), so BN folds into layer-1 weights/bias:
#   [BN(x), A BN(x)] @ W1 + b1 == [x, A x] @ (s_dup*W1) + (b1 + t@(W1a+W1b)).
# A small first dispatch computes per-core partial sums/sumsq of node_feats
# (the full 48-MB reduction stays on device); the host only adds the eight
# partial [640,2] vectors, forms s,t and rescales W1 (0.3 MFLOP of glue).
#
# Layout scheme on device (per block of 123 rows):
#   x_l kept rows-on-partitions ("L_R").  One PE matmul per feature-chunk
#   with rhs = [blockdiag(A_g^T) | I_123 | 0] (256 wide) yields
#   [agg_l^T | x_l^T] (features-on-partitions, "L_F") directly -- the
#   transpose needed by the next dense layer falls out of the same matmul,
#   so the whole GCN chain needs zero explicit transposes.  Dense layers
#   contract features: lhsT = catT chunks (L_F), rhs = W chunks -> x_{l+1}
#   in L_R.  Layer 4 instead uses lhsT = W4 chunks and batches 4 blocks per
#   matmul (N=492) because the tail wants x4^T.  Bias is added with a
#   rank-1 matmul (ones[1,123] x b[1,outs]) into PSUM; ReLU on eviction.
#
# Matmul dtype: float32r (fp32 container, ~13-bit mantissa used by the PE
# at 1 cycle/row for free-dim >= 256, vs 4 cycles/row for plain fp32).
# HW-verified: f32r matmul is exact on pre-rounded inputs; intermediate
# evictions to f32r tiles round once (~1e-4 rel).  End-to-end impact is
# ~2e-4 on gcn_pred; plain-fp32 fallback via KERNEL_FP32=1.
#
# Conv: lhsT = blockdiag over 4 batches of conv_w^T (128x32, cols 24..31
# zero), rhs = pixel chunks [128, 512]; f32r requires PSUM dst partition
# base 0, so two chunks share a 2-bank psum tile side by side, evicted on
# alternating DVE/ACT with per-partition bias.
#
# Block pairs are emitted stage-interleaved so one block's PSUM eviction
# hides behind the other block's matmuls (PE runs its stream in order).
#
# Measured (8x trn2 NeuronCores, via For_i slope): main dispatch ~350 us,
# stats dispatch ~30 us; rel err vs fp32 reference: pred_maps 4.8e-4,
# gcn_pred 9.3e-4.

import os
import sys
import threading

import numpy as np

if "/opt/trn_rl_repo" not in sys.path:
    sys.path.insert(0, "/opt/trn_rl_repo")

import concourse.bass as bass  # noqa: F401
import concourse.tile as tile
from concourse import bacc, mybir

FP32 = mybir.dt.float32
F32R = mybir.dt.float32r
AF = mybir.ActivationFunctionType

USE_F32R = not os.environ.get("KERNEL_FP32")
MMDT = F32R if USE_F32R else FP32

N_CORES = 8
B, C, H, W = 4, 32, 512, 512
G, N, F = 512, 41, 576
CO = 6                     # conv out channels
HS = H // N_CORES          # 64 image rows per core
NPX = HS * W               # 32768 pixels per (core, batch)
GC = G // N_CORES          # 64 graphs per core
GPB = 3                    # graphs per block
NB = 123                   # rows per block (3*41)
NBLK = 22                  # blocks per core (21 full + 1 single-graph)
RPAD = NBLK * NB           # 2706 padded rows per core
ATW = 256                  # per-block [blockdiag(A^T) | I | 0pad] width
FPAD = 640                 # 576 padded to 5*128
EPS = 1e-5
DGRP = [4, 4, 4, 4, 4, 2]  # dense-4 block batching

_lock = threading.Lock()
_cache = {}


def _round_f32r(x):
    """Round fp32 -> the f32r-representable grid (13-bit mantissa)."""
    if not USE_F32R:
        return np.ascontiguousarray(x, np.float32)
    u = np.ascontiguousarray(x, np.float32).view(np.uint32)
    return ((u + np.uint32(0x1000)) & np.uint32(0xFFFFE000)).view(np.float32)


# ----------------------------------------------------------------- builders
def _build_stats_nc():
    """Per-core partial sum / sum-of-squares of node_feats over rows.

    Input  nft  [FPAD, RPAD]  (features on partitions, zero-padded)
    Output stats [FPAD, 2]    (col 0 = sum, col 1 = sumsq)
    """
    nc = bacc.Bacc("TRN2", target_bir_lowering=False, debug=False,
                   num_devices=N_CORES)
    nft = nc.dram_tensor("nft", [FPAD, RPAD], FP32, kind="ExternalInput").ap()
    stats = nc.dram_tensor("stats", [FPAD, 2], FP32, kind="ExternalOutput").ap()
    with tile.TileContext(nc) as tc:
        with (
            tc.tile_pool(name="io", bufs=2) as io,
            tc.tile_pool(name="scratch", bufs=2) as sp,
            tc.tile_pool(name="acc", bufs=10) as ap,
        ):
            for k in range(FPAD // 128):
                t = io.tile([128, RPAD], FP32)
                nc.sync.dma_start(t[:], nft[128 * k:128 * (k + 1), :])
                sums = ap.tile([128, 1], FP32, tag="sums")
                nc.vector.reduce_sum(sums[:], t[:], axis=mybir.AxisListType.X)
                sq = sp.tile([128, RPAD], FP32)
                sqs = ap.tile([128, 1], FP32, tag="sqs")
                nc.scalar.activation(sq[:], t[:], AF.Square, accum_out=sqs[:])
                nc.sync.dma_start(stats[128 * k:128 * (k + 1), 0:1], sums[:])
                nc.sync.dma_start(stats[128 * k:128 * (k + 1), 1:2], sqs[:])
    nc.compile()
    return nc


def _build_main_nc(reps=1):
    nc = bacc.Bacc("TRN2", target_bir_lowering=False, debug=False,
                   num_devices=N_CORES)
    dt = nc.dram_tensor
    aps = {}

    def di(name, shape, dtype=MMDT):
        aps[name] = dt(name, shape, dtype, kind="ExternalInput").ap()

    di("img", [128, NPX])
    di("x0r", [NB, NBLK * FPAD])
    di("atbi", [NB, NBLK * ATW])
    di("w1x", [FPAD, 512]); di("w1a", [FPAD, 512])
    di("w2x", [512, 256]); di("w2a", [512, 256])
    di("w3x", [256, 128]); di("w3a", [256, 128])
    di("w4x", [128, 64]); di("w4a", [128, 64])
    di("b1", [1, 512]); di("b2", [1, 256]); di("b3", [1, 128])
    di("b4", [64, 1], FP32)
    di("cw1", [64, 32], FP32); di("cb1", [32, 1], FP32)
    di("pa", [32, 1], FP32)
    di("cw2", [32, 2], FP32); di("cb2", [2, 1], FP32)
    di("cwbd", [128, 32]); di("cb32", [32, 1], FP32)
    di("ones", [1, NB])
    aps["pred"] = dt("pred", [B, CO, HS, W], FP32, kind="ExternalOutput").ap()
    aps["predt"] = dt("predt", [2, RPAD], FP32, kind="ExternalOutput").ap()

    with tile.TileContext(nc) as tc:
        _emit_main(nc, tc, aps, reps)
    nc.compile()
    return nc


def _emit_main(nc, tc, t, reps):
    from contextlib import ExitStack
    ctx = ExitStack()
    with ctx:
        def pool(name, bufs, space="SBUF"):
            return ctx.enter_context(
                tc.tile_pool(name=name, bufs=bufs, space=space))

        consts = pool("consts", 1)
        imgp = pool("imgp", 2)
        cop = pool("cop", 2)
        x0p = pool("x0p", 4)
        atp = pool("atp", 4)
        blkp = pool("blk", 4)
        c4ap = pool("c4ap", 1)
        x4p = pool("x4p", 1)
        predp = pool("predp", 1)
        hp = pool("hp", 2)
        psb = pool("psb", 3, space="PSUM")    # 2-bank slots (c1/c2/conv)
        pss = pool("pss", 2, space="PSUM")    # 1-bank slots (dense/bmm34)

        # ---- resident constants / weights -------------------------------
        def load_const(ap_in, shape, tag, dtype=MMDT):
            s = consts.tile(shape, dtype, tag=tag)
            nc.sync.dma_start(s[:], ap_in)
            return s

        def load_chunks(name, nchunk, width):
            return [load_const(t[name][128 * k:128 * (k + 1), :],
                               [128, width], f"{name}{k}")
                    for k in range(nchunk)]

        w1x_s = load_chunks("w1x", 5, 512)
        w1a_s = load_chunks("w1a", 5, 512)
        w2x_s = load_chunks("w2x", 4, 256)
        w2a_s = load_chunks("w2a", 4, 256)
        w3x_s = load_chunks("w3x", 2, 128)
        w3a_s = load_chunks("w3a", 2, 128)
        w4x_s = load_const(t["w4x"][:], [128, 64], "w4x")
        w4a_s = load_const(t["w4a"][:], [128, 64], "w4a")
        b1_s = load_const(t["b1"][:], [1, 512], "b1")
        b2_s = load_const(t["b2"][:], [1, 256], "b2")
        b3_s = load_const(t["b3"][:], [1, 128], "b3")
        b4_s = load_const(t["b4"][:], [64, 1], "b4", FP32)
        cw1_s = load_const(t["cw1"][:], [64, 32], "cw1", FP32)
        cb1_s = load_const(t["cb1"][:], [32, 1], "cb1", FP32)
        pa_s = load_const(t["pa"][:], [32, 1], "pa", FP32)
        cw2_s = load_const(t["cw2"][:], [32, 2], "cw2", FP32)
        cb2_s = load_const(t["cb2"][:], [2, 1], "cb2", FP32)
        cwbd_s = load_const(t["cwbd"][:], [128, 32], "cwbd")
        cb32_s = load_const(t["cb32"][:], [32, 1], "cb32", FP32)
        ones_s = load_const(t["ones"][:], [1, NB], "ones")

        # dest view for conv DMA: h split as (sr, j)
        pred_v = t["pred"].rearrange("b o (S j) w -> S (b o) j w", j=4)

        def body(_iv=None):
            # ---- conv emitted as PE stall-filler between GCN stages ----
            # super-round sr covers h-rows 4sr..4sr+3 (chunks j=0..3);
            # f32r matmuls must write PSUM partition base 0, so chunks go
            # side-by-side in the free dim (2 per 2-bank psum tile).
            def conv_round(sr):
                it = imgp.tile([128, 2048], MMDT, tag="img")
                nc.sync.dma_start(
                    it[:], t["img"][:, 2048 * sr:2048 * (sr + 1)])
                out_sb = cop.tile([32, 2048], FP32, tag="convout")
                for half in range(2):
                    ps = psb.tile([32, 1024], FP32, tag="ps")
                    for j2 in range(2):
                        j = 2 * half + j2
                        nc.tensor.matmul(
                            ps[:, 512 * j2:512 * (j2 + 1)],
                            cwbd_s[:, 0:32],
                            it[:, 512 * j:512 * (j + 1)],
                            start=True, stop=True)
                    if half == 0:
                        nc.vector.tensor_scalar_add(
                            out_sb[:, 0:1024], ps[:], cb32_s[:])
                    else:
                        nc.scalar.activation(
                            out_sb[:, 1024:2048], ps[:], AF.Identity,
                            bias=cb32_s[:])
                osv = out_sb.rearrange("p (j w) -> p j w", j=4)
                nc.sync.dma_start(pred_v[sr], osv[0:24])

            conv_pending = list(range(16))

            def maybe_conv(n=1):
                for _ in range(n):
                    if conv_pending:
                        conv_round(conv_pending.pop(0))

            # ---- GCN block chain ---------------------------------------
            c4a_all = c4ap.tile([128, RPAD], MMDT, tag="c4a")
            c4x_all = c4ap.tile([128, RPAD], MMDT, tag="c4x")
            x4t_all = x4p.tile([64, RPAD], FP32, tag="x4t")
            def stage_load(b):
                x0_b = x0p.tile([NB, FPAD], MMDT, tag="x0")
                nc.sync.dma_start(
                    x0_b[:], t["x0r"][:, FPAD * b:FPAD * (b + 1)])
                at_b = atp.tile([NB, ATW], MMDT, tag="at")
                nc.sync.dma_start(
                    at_b[:], t["atbi"][:, ATW * b:ATW * (b + 1)])
                return x0_b, at_b

            def stage_bmm1(b, st):
                x0_b, at_b = st["ld"]
                ps_c1a = psb.tile([128, 1024], FP32, tag="ps")
                ps_c1b = psb.tile([128, 1024], FP32, tag="ps")
                c1 = blkp.tile([128, 1280], MMDT, tag="c1")
                for fc in range(5):
                    pst = ps_c1a if fc < 2 else ps_c1b
                    po = 256 * fc if fc < 2 else 256 * (fc - 2)
                    nc.tensor.matmul(
                        pst[:, po:po + 256],
                        x0_b[:, 128 * fc:128 * (fc + 1)],
                        at_b[:], start=True, stop=True)
                nc.vector.tensor_copy(c1[:, 0:512], ps_c1a[:, 0:512])
                nc.vector.tensor_copy(c1[:, 512:1280], ps_c1b[:, 0:768])
                st["c1"] = c1

            def stage_dense1(b, st):
                c1 = st["c1"]
                ps_x1 = pss.tile([NB, 512], FP32, tag="ps")
                for k in range(5):
                    nc.tensor.matmul(
                        ps_x1[:], c1[:, 256 * k + NB:256 * k + 2 * NB],
                        w1x_s[k][:], start=(k == 0), stop=False)
                for k in range(5):
                    nc.tensor.matmul(
                        ps_x1[:], c1[:, 256 * k:256 * k + NB],
                        w1a_s[k][:], start=False, stop=False)
                nc.tensor.matmul(ps_x1[:], ones_s[:], b1_s[:],
                                 start=False, stop=True)
                x1 = blkp.tile([NB, 512], MMDT, tag="x1")
                nc.scalar.activation(x1[:], ps_x1[:], AF.Relu)
                st["x1"] = x1

            def stage_bmm2(b, st):
                at_b = st["ld"][1]
                x1 = st["x1"]
                ps_c2 = psb.tile([128, 1024], FP32, tag="ps")
                for fc in range(4):
                    nc.tensor.matmul(
                        ps_c2[:, 256 * fc:256 * (fc + 1)],
                        x1[:, 128 * fc:128 * (fc + 1)],
                        at_b[:], start=True, stop=True)
                c2 = blkp.tile([128, 1024], MMDT, tag="c2")
                nc.scalar.copy(c2[:], ps_c2[:])
                st["c2"] = c2

            def stage_dense2(b, st):
                c2 = st["c2"]
                ps_x2 = pss.tile([NB, 256], FP32, tag="ps")
                for k in range(4):
                    nc.tensor.matmul(
                        ps_x2[:], c2[:, 256 * k + NB:256 * k + 2 * NB],
                        w2x_s[k][:], start=(k == 0), stop=False)
                for k in range(4):
                    nc.tensor.matmul(
                        ps_x2[:], c2[:, 256 * k:256 * k + NB],
                        w2a_s[k][:], start=False, stop=False)
                nc.tensor.matmul(ps_x2[:], ones_s[:], b2_s[:],
                                 start=False, stop=True)
                x2 = blkp.tile([NB, 256], MMDT, tag="x2")
                nc.scalar.activation(x2[:], ps_x2[:], AF.Relu)
                st["x2"] = x2

            def stage_bmm3(b, st):
                at_b = st["ld"][1]
                x2 = st["x2"]
                ps_c3 = pss.tile([128, 512], FP32, tag="ps")
                for fc in range(2):
                    nc.tensor.matmul(
                        ps_c3[:, 256 * fc:256 * (fc + 1)],
                        x2[:, 128 * fc:128 * (fc + 1)],
                        at_b[:], start=True, stop=True)
                c3 = blkp.tile([128, 512], MMDT, tag="c3")
                nc.vector.tensor_copy(c3[:], ps_c3[:])
                st["c3"] = c3

            def stage_dense3(b, st):
                c3 = st["c3"]
                ps_x3 = pss.tile([NB, 128], FP32, tag="ps")
                for k in range(2):
                    nc.tensor.matmul(
                        ps_x3[:], c3[:, 256 * k + NB:256 * k + 2 * NB],
                        w3x_s[k][:], start=(k == 0), stop=False)
                for k in range(2):
                    nc.tensor.matmul(
                        ps_x3[:], c3[:, 256 * k:256 * k + NB],
                        w3a_s[k][:], start=False, stop=False)
                nc.tensor.matmul(ps_x3[:], ones_s[:], b3_s[:],
                                 start=False, stop=True)
                x3 = blkp.tile([NB, 128], MMDT, tag="x3")
                nc.scalar.activation(x3[:], ps_x3[:], AF.Relu)
                st["x3"] = x3

            def stage_bmm4(b, st):
                at_b = st["ld"][1]
                x3 = st["x3"]
                ps_c4 = pss.tile([128, ATW], FP32, tag="ps")
                nc.tensor.matmul(ps_c4[:], x3[:], at_b[:],
                                 start=True, stop=True)
                nc.vector.tensor_copy(
                    c4a_all[:, NB * b:NB * (b + 1)], ps_c4[:, 0:NB])
                nc.vector.tensor_copy(
                    c4x_all[:, NB * b:NB * (b + 1)], ps_c4[:, NB:2 * NB])

            stages = [stage_bmm1, stage_dense1, stage_bmm2, stage_dense2,
                      stage_bmm3, stage_dense3, stage_bmm4]
            # emit pairs of blocks stage-interleaved so one block's PSUM
            # eviction hides behind the other block's matmuls (PE executes
            # its stream in emission order)
            maybe_conv(2)
            nstage = 0
            groups = [list(range(4 * p, 4 * p + 4)) for p in range(5)]
            groups.append([20, 21])
            for grp in groups:
                sts = [{"ld": stage_load(b)} for b in grp]
                for stg in stages:
                    for b, st in zip(grp, sts):
                        stg(b, st)
                    nstage += 1
                    if nstage % 4 == 0:
                        maybe_conv(1)
            maybe_conv(16)

            # dense4 (option B), 4 blocks per matmul: x4T = relu(W4.T catT+b4)
            b0 = 0
            for nb in DGRP:
                ps_x4 = pss.tile([64, 4 * NB], FP32, tag="ps")
                wv = nb * NB
                nc.tensor.matmul(ps_x4[:, 0:wv],
                                 w4x_s[:], c4x_all[:, NB * b0:NB * b0 + wv],
                                 start=True, stop=False)
                nc.tensor.matmul(ps_x4[:, 0:wv],
                                 w4a_s[:], c4a_all[:, NB * b0:NB * b0 + wv],
                                 start=False, stop=True)
                nc.scalar.activation(
                    x4t_all[:, NB * b0:NB * b0 + wv], ps_x4[:, 0:wv],
                    AF.Relu, bias=b4_s[:])
                b0 += nb

            # ---- classifier tail on x4T --------------------------------
            predt_sb = predp.tile([2, RPAD], FP32, tag="predt")
            CH = [512] * 5 + [RPAD - 5 * 512]
            off = 0
            for w in CH:
                ps_h = pss.tile([32, 512], FP32, tag="ps")
                nc.tensor.matmul(ps_h[:, 0:w], cw1_s[:],
                                 x4t_all[:, off:off + w],
                                 start=True, stop=True)
                # PReLU(z) = max(z,0) + a*min(z,0), z = W1h@x4 + b1h
                z_sb = hp.tile([32, 512], FP32, tag="z")
                nc.scalar.activation(z_sb[:, 0:w], ps_h[:, 0:w], AF.Identity,
                                     bias=cb1_s[:])
                hneg = hp.tile([32, 512], FP32, tag="hneg")
                nc.vector.tensor_scalar(hneg[:, 0:w], z_sb[:, 0:w], 0.0,
                                        pa_s[:], mybir.AluOpType.min,
                                        mybir.AluOpType.mult)
                h_sb = hp.tile([32, 512], FP32, tag="h")
                nc.vector.tensor_scalar_max(h_sb[:, 0:w], z_sb[:, 0:w], 0.0)
                nc.vector.tensor_add(h_sb[:, 0:w], h_sb[:, 0:w],
                                     hneg[:, 0:w])
                ps_p = pss.tile([2, 512], FP32, tag="ps")
                nc.tensor.matmul(ps_p[:, 0:w], cw2_s[:], h_sb[:, 0:w],
                                 start=True, stop=True)
                nc.scalar.activation(predt_sb[:, off:off + w], ps_p[:, 0:w],
                                     AF.Identity, bias=cb2_s[:])
                off += w
            nc.sync.dma_start(t["predt"][:], predt_sb[:])

        if reps == 1:
            body()
        else:
            with tc.For_i(0, reps, 1) as iv:
                body(iv)


# ------------------------------------------------------------- host prep
def _pad_rows(sh):
    """[GC*N, F] -> [RPAD, F] with zero pad rows per 3-graph block."""
    rp = np.zeros((RPAD, F), np.float32)
    full = (GC * N) // (GPB * N)          # 21 full blocks
    rp[:full * NB] = sh[:full * GPB * N].reshape(full * NB, F)
    rem = GC * N - full * GPB * N
    if rem:
        rp[full * NB:full * NB + rem] = sh[full * GPB * N:]
    return rp


def _prep_stats_inputs(node_feats):
    nf = np.ascontiguousarray(node_feats, np.float32).reshape(G, N, F)
    maps = []
    for c in range(N_CORES):
        rp = _pad_rows(nf[c * GC:(c + 1) * GC].reshape(GC * N, F))
        nft = np.zeros((FPAD, RPAD), np.float32)
        nft[:F, :] = rp.T
        maps.append({"nft": nft})
    return maps


def _prep_main_inputs(inputs, node_feats, A, conv_w, conv_b, w1s, b1s,
                      W2, b2v, W3, b3v, W4, b4v,
                      cls_w1, cls_b1, prelu_a, cls_w2, cls_b2):
    f32 = lambda x: np.ascontiguousarray(x, np.float32)
    R = _round_f32r
    inputs, node_feats, A = f32(inputs), f32(node_feats), f32(A)
    cwbd = np.zeros((128, 32), np.float32)
    cb32 = np.zeros((32, 1), np.float32)
    wt = f32(conv_w).T  # [32, 6]
    for b in range(B):
        cwbd[32 * b:32 * (b + 1), 6 * b:6 * (b + 1)] = wt
        cb32[6 * b:6 * (b + 1), 0] = f32(conv_b)
    w1xp = np.zeros((FPAD, 512), np.float32)
    w1ap = np.zeros((FPAD, 512), np.float32)
    w1xp[:F] = w1s[:F]
    w1ap[:F] = w1s[F:]
    shared = {
        "w1x": R(w1xp), "w1a": R(w1ap),
        "w2x": R(W2[:512]), "w2a": R(W2[512:]),
        "w3x": R(W3[:256]), "w3a": R(W3[256:]),
        "w4x": R(W4[:128]), "w4a": R(W4[128:]),
        "b1": R(f32(b1s).reshape(1, 512)), "b2": R(f32(b2v).reshape(1, 256)),
        "b3": R(f32(b3v).reshape(1, 128)), "b4": f32(b4v).reshape(64, 1),
        "cw1": f32(cls_w1), "cb1": f32(cls_b1).reshape(32, 1),
        "pa": f32(prelu_a).reshape(32, 1), "cw2": f32(cls_w2),
        "cb2": f32(cls_b2).reshape(2, 1),
        "cwbd": R(cwbd), "cb32": cb32,
        "ones": np.ones((1, NB), np.float32),
    }
    # atbi: [123, NBLK*256]: per block [blockdiag(A_g^T) | I | 0]
    At = A.transpose(0, 2, 1)             # [G, 41, 41] = A^T per graph
    eye = np.eye(NB, dtype=np.float32)
    maps = []
    for c in range(N_CORES):
        img = inputs[:, :, c * HS:(c + 1) * HS, :].reshape(128, NPX)
        rp = _pad_rows(node_feats[c * GC:(c + 1) * GC].reshape(GC * N, F))
        x0r = np.zeros((NB, NBLK, FPAD), np.float32)
        x0r[:, :, :F] = rp.reshape(NBLK, NB, F).transpose(1, 0, 2)
        atbi = np.zeros((NB, NBLK, ATW), np.float32)
        Ash = At[c * GC:(c + 1) * GC]
        for b in range(NBLK):
            ng = min(GPB, GC - b * GPB)
            for gi in range(ng):
                atbi[N * gi:N * (gi + 1), b, N * gi:N * (gi + 1)] = \
                    Ash[b * GPB + gi]
            atbi[:, b, NB:2 * NB] = eye
        m = dict(shared)
        m.update({"img": R(img), "x0r": R(x0r.reshape(NB, NBLK * FPAD)),
                  "atbi": R(atbi.reshape(NB, NBLK * ATW))})
        maps.append(m)
    return maps


# ------------------------------------------------------------- execution
def _output_names(nc):
    names = []
    for alloc in nc.m.functions[0].allocations:
        if isinstance(alloc, mybir.MemoryLocationSet) and \
                alloc.kind == "ExternalOutput":
            names.append(alloc.memorylocations[0].name)
    return names


def _run(nc, in_maps):
    if os.environ.get("KERNEL_SIM"):
        from concourse.bass_interp import MultiCoreSim
        sim = MultiCoreSim(nc, num_cores=N_CORES, trace=False)
        for c in range(N_CORES):
            for k, v in in_maps[c].items():
                sim.cores[c].tensor(k)[:] = v
        sim.simulate()
        onames = _output_names(nc)
        return [{n: np.array(sim.cores[c].tensor(n)) for n in onames}
                for c in range(N_CORES)]
    from concourse.bass_utils import run_bass_kernel_spmd
    return run_bass_kernel_spmd(nc, in_maps, list(range(N_CORES))).results


def _get_nc(which, reps=1):
    key = (which, reps)
    with _lock:
        if key not in _cache:
            if which == "stats":
                _cache[key] = _build_stats_nc()
            else:
                _cache[key] = _build_main_nc(reps)
        return _cache[key]


def kernel(inputs, node_feats, A, knn_inx, conv_w, conv_b, W1, b1, W2, b2,
           W3, b3, W4, b4, cls_w1, cls_b1, prelu_a, cls_w2, cls_b2):
    f32 = lambda x: np.asarray(x, np.float32)
    # ---- dispatch A: BN statistics ------------------------------------
    nc_s = _get_nc("stats")
    smaps = _prep_stats_inputs(node_feats)
    sres = _run(nc_s, smaps)
    tot = np.zeros((FPAD, 2), np.float64)
    for c in range(N_CORES):
        tot += sres[c]["stats"].astype(np.float64)
    cnt = float(G * N)
    mean = (tot[:F, 0] / cnt).astype(np.float32)
    var = (tot[:F, 1] / cnt).astype(np.float32) - mean * mean
    s = (1.0 / np.sqrt(var + np.float32(EPS))).astype(np.float32)
    t_ = (-mean * s).astype(np.float32)
    W1 = f32(W1)
    sdup = np.concatenate([s, s]).astype(np.float32)
    w1s = W1 * sdup[:, None]
    b1s = f32(b1) + t_ @ W1[:F] + t_ @ W1[F:]

    # ---- dispatch B: conv + GCN ---------------------------------------
    nc_m = _get_nc("main")
    mmaps = _prep_main_inputs(inputs, node_feats, A, conv_w, conv_b,
                              w1s, b1s, W2, b2, W3, b3, W4, b4,
                              cls_w1, cls_b1, prelu_a, cls_w2, cls_b2)
    mres = _run(nc_m, mmaps)

    # ---- host gather/unshard ------------------------------------------
    pred_maps = np.concatenate(
        [mres[c]["pred"] for c in range(N_CORES)], axis=2)
    x4pred = np.zeros((G * N, 2), np.float32)
    for c in range(N_CORES):
        pt = mres[c]["predt"]  # [2, RPAD]
        sh = pt.T.reshape(NBLK, NB, 2)
        full = (GC * N) // (GPB * N)
        rem = GC * N - full * GPB * N
        rows = [sh[:full].reshape(full * NB, 2)]
        if rem:
            rows.append(sh[full, :rem])
        x4pred[c * GC * N:(c + 1) * GC * N] = np.concatenate(rows, 0)
    x4pred = x4pred.reshape(G, N, 2)
    ki = np.asarray(knn_inx)
    gcn_pred = np.take_along_axis(
        x4pred, ki[:, :, None].astype(np.int64), axis=1).reshape(-1, 2)
    return pred_maps, gcn_pred


# revision 18
# speedup vs baseline: 370.8045x; 1.0515x over previous

# Trainium2 Bass kernel for nn_DRRGHead (1x1 conv head + 4-layer GraphConv GCN
# over 512 independent local graphs + knn-gather classifier tail).
#
# Sharding (8 cores, data-parallel):
#   - image: H=512 split into 8 slabs of 64 rows; each core convolves
#     (4 batches x 32 ch x 64 rows x 512 cols) with the 6x32 1x1-conv.
#   - graphs: G=512 split into 64 graphs/core, processed as 22 blocks of
#     3 graphs (3*41=123 rows on partitions; last block 1 graph, zero-padded).
#
# BatchNorm handling: BN(x) = x*s + t is affine and A is row-normalized
# (A @ 1 == 1), so BN folds into layer-1 weights/bias:
#   [BN(x), A BN(x)] @ W1 + b1 == [x, A x] @ (s_dup*W1) + (b1 + t@(W1a+W1b)).
# A small first dispatch computes per-core partial sums/sumsq of node_feats
# (the full 48-MB reduction stays on device); the host only adds the eight
# partial [640,2] vectors, forms s,t and rescales W1 (0.3 MFLOP of glue).
#
# Layout scheme on device (per block of 123 rows):
#   x_l kept rows-on-partitions ("L_R").  One PE matmul per feature-chunk
#   with rhs = [blockdiag(A_g^T) | I_123 | 0] (256 wide) yields
#   [agg_l^T | x_l^T] (features-on-partitions, "L_F") directly -- the
#   transpose needed by the next dense layer falls out of the same matmul,
#   so the whole GCN chain needs zero explicit transposes.  Dense layers
#   contract features: lhsT = catT chunks (L_F), rhs = W chunks -> x_{l+1}
#   in L_R.  Layer 4 instead uses lhsT = W4 chunks and batches 4 blocks per
#   matmul (N=492) because the tail wants x4^T.  Bias is added with a
#   rank-1 matmul (ones[1,123] x b[1,outs]) into PSUM; ReLU on eviction.
#
# Matmul dtype: float32r (fp32 container, ~13-bit mantissa used by the PE
# at 1 cycle/row for free-dim >= 256, vs 4 cycles/row for plain fp32).
# HW-verified: f32r matmul is exact on pre-rounded inputs; intermediate
# evictions to f32r tiles round once (~1e-4 rel).  End-to-end impact is
# ~2e-4 on gcn_pred; plain-fp32 fallback via KERNEL_FP32=1.
#
# Conv: lhsT = blockdiag over 4 batches of conv_w^T (128x32, cols 24..31
# zero), rhs = pixel chunks [128, 512]; f32r requires PSUM dst partition
# base 0, so two chunks share a 2-bank psum tile side by side, evicted on
# alternating DVE/ACT with per-partition bias.
#
# Blocks are emitted in stage-interleaved groups of 4 so one block's PSUM
# eviction hides behind the other blocks' matmuls (PE runs its stream in
# emission order), and conv super-rounds are spliced between GCN stages as
# additional PE stall-filler.  PSUM: one 2-bank pool (bufs=3) for the wide
# bmm tiles + one 1-bank pool (bufs=2) for dense/eviction tiles = 8 banks.
#
# Measured (8x trn2 NeuronCores, via For_i slope): main dispatch ~289 us
# (serial-block version was 463 us), stats dispatch ~30 us; rel err vs
# fp32 reference: pred_maps 4.8e-4, gcn_pred 9.3e-4.

import os
import sys
import threading

import numpy as np

if "/opt/trn_rl_repo" not in sys.path:
    sys.path.insert(0, "/opt/trn_rl_repo")

import concourse.bass as bass  # noqa: F401
import concourse.tile as tile
from concourse import bacc, mybir

FP32 = mybir.dt.float32
F32R = mybir.dt.float32r
AF = mybir.ActivationFunctionType

USE_F32R = not os.environ.get("KERNEL_FP32")
MMDT = F32R if USE_F32R else FP32

N_CORES = 8
B, C, H, W = 4, 32, 512, 512
G, N, F = 512, 41, 576
CO = 6                     # conv out channels
HS = H // N_CORES          # 64 image rows per core
NPX = HS * W               # 32768 pixels per (core, batch)
GC = G // N_CORES          # 64 graphs per core
GPB = 3                    # graphs per block
NB = 123                   # rows per block (3*41)
NBLK = 22                  # blocks per core (21 full + 1 single-graph)
RPAD = NBLK * NB           # 2706 padded rows per core
ATW = 256                  # per-block [blockdiag(A^T) | I | 0pad] width
FPAD = 640                 # 576 padded to 5*128
EPS = 1e-5
DGRP = [4, 4, 4, 4, 4, 2]  # dense-4 block batching

_lock = threading.Lock()
_cache = {}


def _round_f32r(x):
    """Round fp32 -> the f32r-representable grid (13-bit mantissa)."""
    if not USE_F32R:
        return np.ascontiguousarray(x, np.float32)
    u = np.ascontiguousarray(x, np.float32).view(np.uint32)
    return ((u + np.uint32(0x1000)) & np.uint32(0xFFFFE000)).view(np.float32)


# ----------------------------------------------------------------- builders
def _build_stats_nc():
    """Per-core partial sum / sum-of-squares of node_feats over rows.

    Input  nft  [FPAD, RPAD]  (features on partitions, zero-padded)
    Output stats [FPAD, 2]    (col 0 = sum, col 1 = sumsq)
    """
    nc = bacc.Bacc("TRN2", target_bir_lowering=False, debug=False,
                   num_devices=N_CORES)
    nft = nc.dram_tensor("nft", [FPAD, RPAD], FP32, kind="ExternalInput").ap()
    stats = nc.dram_tensor("stats", [FPAD, 2], FP32, kind="ExternalOutput").ap()
    with tile.TileContext(nc) as tc:
        with (
            tc.tile_pool(name="io", bufs=2) as io,
            tc.tile_pool(name="scratch", bufs=2) as sp,
            tc.tile_pool(name="acc", bufs=10) as ap,
        ):
            for k in range(FPAD // 128):
                t = io.tile([128, RPAD], FP32)
                nc.sync.dma_start(t[:], nft[128 * k:128 * (k + 1), :])
                sums = ap.tile([128, 1], FP32, tag="sums")
                nc.vector.reduce_sum(sums[:], t[:], axis=mybir.AxisListType.X)
                sq = sp.tile([128, RPAD], FP32)
                sqs = ap.tile([128, 1], FP32, tag="sqs")
                nc.scalar.activation(sq[:], t[:], AF.Square, accum_out=sqs[:])
                nc.sync.dma_start(stats[128 * k:128 * (k + 1), 0:1], sums[:])
                nc.sync.dma_start(stats[128 * k:128 * (k + 1), 1:2], sqs[:])
    nc.compile()
    return nc


def _build_main_nc(reps=1):
    nc = bacc.Bacc("TRN2", target_bir_lowering=False, debug=False,
                   num_devices=N_CORES)
    dt = nc.dram_tensor
    aps = {}

    def di(name, shape, dtype=MMDT):
        aps[name] = dt(name, shape, dtype, kind="ExternalInput").ap()

    di("img", [128, NPX])
    di("x0r", [NB, NBLK * FPAD])
    di("atbi", [NB, NBLK * ATW])
    di("w1x", [FPAD, 512]); di("w1a", [FPAD, 512])
    di("w2x", [512, 256]); di("w2a", [512, 256])
    di("w3x", [256, 128]); di("w3a", [256, 128])
    di("w4x", [128, 64]); di("w4a", [128, 64])
    di("b1", [1, 512]); di("b2", [1, 256]); di("b3", [1, 128])
    di("b4", [64, 1], FP32)
    di("cw1", [64, 32]); di("cb1", [32, 1], FP32)
    di("pa", [32, 1], FP32)
    di("cw2", [32, 2], FP32); di("cb2", [2, 1], FP32)
    di("cwbd", [128, 32]); di("cb32", [32, 1], FP32)
    di("ones", [1, NB])
    aps["pred"] = dt("pred", [B, CO, HS, W], FP32, kind="ExternalOutput").ap()
    aps["predt"] = dt("predt", [2, RPAD], FP32, kind="ExternalOutput").ap()

    with tile.TileContext(nc) as tc:
        _emit_main(nc, tc, aps, reps)
    nc.compile()
    return nc


def _emit_main(nc, tc, t, reps):
    from contextlib import ExitStack
    ctx = ExitStack()
    with ctx:
        def pool(name, bufs, space="SBUF"):
            return ctx.enter_context(
                tc.tile_pool(name=name, bufs=bufs, space=space))

        consts = pool("consts", 1)
        imgp = pool("imgp", 2)
        cop = pool("cop", 2)
        x0p = pool("x0p", 4)
        atp = pool("atp", 4)
        blkp = pool("blk", 4)
        c4ap = pool("c4ap", 1)
        x4p = pool("x4p", 1)
        predp = pool("predp", 1)
        hp = pool("hp", 2)
        psb = pool("psb", 3, space="PSUM")    # 2-bank slots (c1/c2/conv)
        pss = pool("pss", 2, space="PSUM")    # 1-bank slots (dense/bmm34)

        # ---- resident constants / weights -------------------------------
        def load_const(ap_in, shape, tag, dtype=MMDT):
            s = consts.tile(shape, dtype, tag=tag)
            nc.sync.dma_start(s[:], ap_in)
            return s

        def load_chunks(name, nchunk, width):
            return [load_const(t[name][128 * k:128 * (k + 1), :],
                               [128, width], f"{name}{k}")
                    for k in range(nchunk)]

        w1x_s = load_chunks("w1x", 5, 512)
        w1a_s = load_chunks("w1a", 5, 512)
        w2x_s = load_chunks("w2x", 4, 256)
        w2a_s = load_chunks("w2a", 4, 256)
        w3x_s = load_chunks("w3x", 2, 128)
        w3a_s = load_chunks("w3a", 2, 128)
        w4x_s = load_const(t["w4x"][:], [128, 64], "w4x")
        w4a_s = load_const(t["w4a"][:], [128, 64], "w4a")
        b1_s = load_const(t["b1"][:], [1, 512], "b1")
        b2_s = load_const(t["b2"][:], [1, 256], "b2")
        b3_s = load_const(t["b3"][:], [1, 128], "b3")
        b4_s = load_const(t["b4"][:], [64, 1], "b4", FP32)
        cw1_s = load_const(t["cw1"][:], [64, 32], "cw1")
        cb1_s = load_const(t["cb1"][:], [32, 1], "cb1", FP32)
        pa_s = load_const(t["pa"][:], [32, 1], "pa", FP32)
        cw2_s = load_const(t["cw2"][:], [32, 2], "cw2", FP32)
        cb2_s = load_const(t["cb2"][:], [2, 1], "cb2", FP32)
        cwbd_s = load_const(t["cwbd"][:], [128, 32], "cwbd")
        cb32_s = load_const(t["cb32"][:], [32, 1], "cb32", FP32)
        ones_s = load_const(t["ones"][:], [1, NB], "ones")

        # dest view for conv DMA: h split as (sr, j)
        pred_v = t["pred"].rearrange("b o (S j) w -> S (b o) j w", j=4)

        def body(_iv=None):
            # ---- conv emitted as PE stall-filler between GCN stages ----
            # super-round sr covers h-rows 4sr..4sr+3 (chunks j=0..3);
            # f32r matmuls must write PSUM partition base 0, so chunks go
            # side-by-side in the free dim (2 per 2-bank psum tile).
            def conv_round(sr):
                it = imgp.tile([128, 2048], MMDT, tag="img")
                nc.sync.dma_start(
                    it[:], t["img"][:, 2048 * sr:2048 * (sr + 1)])
                out_sb = cop.tile([32, 2048], FP32, tag="convout")
                for half in range(2):
                    ps = psb.tile([32, 1024], FP32, tag="ps")
                    for j2 in range(2):
                        j = 2 * half + j2
                        nc.tensor.matmul(
                            ps[:, 512 * j2:512 * (j2 + 1)],
                            cwbd_s[:, 0:32],
                            it[:, 512 * j:512 * (j + 1)],
                            start=True, stop=True)
                    if half == 0:
                        nc.vector.tensor_scalar_add(
                            out_sb[:, 0:1024], ps[:], cb32_s[:])
                    else:
                        nc.scalar.activation(
                            out_sb[:, 1024:2048], ps[:], AF.Identity,
                            bias=cb32_s[:])
                osv = out_sb.rearrange("p (j w) -> p j w", j=4)
                nc.sync.dma_start(pred_v[sr], osv[0:24])

            conv_pending = list(range(16))

            def maybe_conv(n=1):
                for _ in range(n):
                    if conv_pending:
                        conv_round(conv_pending.pop(0))

            # ---- GCN block chain ---------------------------------------
            c4a_all = c4ap.tile([128, RPAD], MMDT, tag="c4a")
            c4x_all = c4ap.tile([128, RPAD], MMDT, tag="c4x")
            x4t_all = x4p.tile([64, RPAD], MMDT, tag="x4t")
            def stage_load(b):
                x0_b = x0p.tile([NB, FPAD], MMDT, tag="x0")
                nc.sync.dma_start(
                    x0_b[:], t["x0r"][:, FPAD * b:FPAD * (b + 1)])
                at_b = atp.tile([NB, ATW], MMDT, tag="at")
                nc.sync.dma_start(
                    at_b[:], t["atbi"][:, ATW * b:ATW * (b + 1)])
                return x0_b, at_b

            def stage_bmm1(b, st):
                x0_b, at_b = st["ld"]
                ps_c1a = psb.tile([128, 1024], FP32, tag="ps")
                ps_c1b = psb.tile([128, 1024], FP32, tag="ps")
                c1 = blkp.tile([128, 1280], MMDT, tag="c1")
                for fc in range(5):
                    pst = ps_c1a if fc < 2 else ps_c1b
                    po = 256 * fc if fc < 2 else 256 * (fc - 2)
                    nc.tensor.matmul(
                        pst[:, po:po + 256],
                        x0_b[:, 128 * fc:128 * (fc + 1)],
                        at_b[:], start=True, stop=True)
                nc.vector.tensor_copy(c1[:, 0:512], ps_c1a[:, 0:512])
                nc.vector.tensor_copy(c1[:, 512:1280], ps_c1b[:, 0:768])
                st["c1"] = c1

            def stage_dense1(b, st):
                c1 = st["c1"]
                ps_x1 = pss.tile([NB, 512], FP32, tag="ps")
                for k in range(5):
                    nc.tensor.matmul(
                        ps_x1[:], c1[:, 256 * k + NB:256 * k + 2 * NB],
                        w1x_s[k][:], start=(k == 0), stop=False)
                for k in range(5):
                    nc.tensor.matmul(
                        ps_x1[:], c1[:, 256 * k:256 * k + NB],
                        w1a_s[k][:], start=False, stop=(k == 4))
                x1 = blkp.tile([NB, 512], MMDT, tag="x1")
                nc.scalar.activation(x1[:], ps_x1[:], AF.Relu)
                st["x1"] = x1

            def stage_bmm2(b, st):
                at_b = st["ld"][1]
                x1 = st["x1"]
                ps_c2 = psb.tile([128, 1024], FP32, tag="ps")
                for fc in range(4):
                    nc.tensor.matmul(
                        ps_c2[:, 256 * fc:256 * (fc + 1)],
                        x1[:, 128 * fc:128 * (fc + 1)],
                        at_b[:], start=True, stop=True)
                c2 = blkp.tile([128, 1024], MMDT, tag="c2")
                nc.scalar.copy(c2[:], ps_c2[:])
                st["c2"] = c2

            def stage_dense2(b, st):
                c2 = st["c2"]
                ps_x2 = pss.tile([NB, 256], FP32, tag="ps")
                for k in range(4):
                    nc.tensor.matmul(
                        ps_x2[:], c2[:, 256 * k + NB:256 * k + 2 * NB],
                        w2x_s[k][:], start=(k == 0), stop=False)
                for k in range(4):
                    nc.tensor.matmul(
                        ps_x2[:], c2[:, 256 * k:256 * k + NB],
                        w2a_s[k][:], start=False, stop=False)
                nc.tensor.matmul(ps_x2[:], ones_s[:], b2_s[:],
                                 start=False, stop=True)
                x2 = blkp.tile([NB, 256], MMDT, tag="x2")
                nc.scalar.activation(x2[:], ps_x2[:], AF.Relu)
                st["x2"] = x2

            def stage_bmm3(b, st):
                at_b = st["ld"][1]
                x2 = st["x2"]
                ps_c3 = pss.tile([128, 512], FP32, tag="ps")
                for fc in range(2):
                    nc.tensor.matmul(
                        ps_c3[:, 256 * fc:256 * (fc + 1)],
                        x2[:, 128 * fc:128 * (fc + 1)],
                        at_b[:], start=True, stop=True)
                c3 = blkp.tile([128, 512], MMDT, tag="c3")
                nc.vector.tensor_copy(c3[:], ps_c3[:])
                st["c3"] = c3

            def stage_dense3(b, st):
                c3 = st["c3"]
                ps_x3 = pss.tile([NB, 128], FP32, tag="ps")
                for k in range(2):
                    nc.tensor.matmul(
                        ps_x3[:], c3[:, 256 * k + NB:256 * k + 2 * NB],
                        w3x_s[k][:], start=(k == 0), stop=False)
                for k in range(2):
                    nc.tensor.matmul(
                        ps_x3[:], c3[:, 256 * k:256 * k + NB],
                        w3a_s[k][:], start=False, stop=False)
                nc.tensor.matmul(ps_x3[:], ones_s[:], b3_s[:],
                                 start=False, stop=True)
                x3 = blkp.tile([NB, 128], MMDT, tag="x3")
                nc.scalar.activation(x3[:], ps_x3[:], AF.Relu)
                st["x3"] = x3

            def stage_bmm4(b, st):
                at_b = st["ld"][1]
                x3 = st["x3"]
                ps_c4 = pss.tile([128, ATW], FP32, tag="ps")
                nc.tensor.matmul(ps_c4[:], x3[:], at_b[:],
                                 start=True, stop=True)
                nc.vector.tensor_copy(
                    c4a_all[:, NB * b:NB * (b + 1)], ps_c4[:, 0:NB])
                nc.vector.tensor_copy(
                    c4x_all[:, NB * b:NB * (b + 1)], ps_c4[:, NB:2 * NB])

            stages = [stage_bmm1, stage_dense1, stage_bmm2, stage_dense2,
                      stage_bmm3, stage_dense3, stage_bmm4]
            # emit pairs of blocks stage-interleaved so one block's PSUM
            # eviction hides behind the other block's matmuls (PE executes
            # its stream in emission order)
            maybe_conv(2)
            nstage = 0
            groups = [list(range(4 * p, 4 * p + 4)) for p in range(5)]
            groups.append([20, 21])
            for grp in groups:
                sts = [{"ld": stage_load(b)} for b in grp]
                for stg in stages:
                    for b, st in zip(grp, sts):
                        stg(b, st)
                    nstage += 1
                    if nstage % 4 == 0:
                        maybe_conv(1)
            maybe_conv(16)

            # dense4 (option B), 4 blocks per matmul: x4T = relu(W4.T catT+b4)
            b0 = 0
            for nb in DGRP:
                ps_x4 = pss.tile([64, 4 * NB], FP32, tag="ps")
                wv = nb * NB
                nc.tensor.matmul(ps_x4[:, 0:wv],
                                 w4x_s[:], c4x_all[:, NB * b0:NB * b0 + wv],
                                 start=True, stop=False)
                nc.tensor.matmul(ps_x4[:, 0:wv],
                                 w4a_s[:], c4a_all[:, NB * b0:NB * b0 + wv],
                                 start=False, stop=True)
                nc.scalar.activation(
                    x4t_all[:, NB * b0:NB * b0 + wv], ps_x4[:, 0:wv],
                    AF.Relu, bias=b4_s[:])
                b0 += nb

            # ---- classifier tail on x4T --------------------------------
            predt_sb = predp.tile([2, RPAD], FP32, tag="predt")
            CH = [512] * 5 + [RPAD - 5 * 512]
            off = 0
            for w in CH:
                ps_h = pss.tile([32, 512], FP32, tag="ps")
                nc.tensor.matmul(ps_h[:, 0:w], cw1_s[:],
                                 x4t_all[:, off:off + w],
                                 start=True, stop=True)
                # PReLU(z) = max(z,0) + a*min(z,0), z = W1h@x4 + b1h
                z_sb = hp.tile([32, 512], FP32, tag="z")
                nc.scalar.activation(z_sb[:, 0:w], ps_h[:, 0:w], AF.Identity,
                                     bias=cb1_s[:])
                hneg = hp.tile([32, 512], FP32, tag="hneg")
                nc.vector.tensor_scalar(hneg[:, 0:w], z_sb[:, 0:w], 0.0,
                                        pa_s[:], mybir.AluOpType.min,
                                        mybir.AluOpType.mult)
                h_sb = hp.tile([32, 512], FP32, tag="h")
                nc.vector.tensor_scalar_max(h_sb[:, 0:w], z_sb[:, 0:w], 0.0)
                nc.vector.tensor_add(h_sb[:, 0:w], h_sb[:, 0:w],
                                     hneg[:, 0:w])
                ps_p = pss.tile([2, 512], FP32, tag="ps")
                nc.tensor.matmul(ps_p[:, 0:w], cw2_s[:], h_sb[:, 0:w],
                                 start=True, stop=True)
                nc.scalar.activation(predt_sb[:, off:off + w], ps_p[:, 0:w],
                                     AF.Identity, bias=cb2_s[:])
                off += w
            nc.sync.dma_start(t["predt"][:], predt_sb[:])

        if reps == 1:
            body()
        else:
            with tc.For_i(0, reps, 1) as iv:
                body(iv)


# ------------------------------------------------------------- host prep
def _pad_rows(sh):
    """[GC*N, F] -> [RPAD, F] with zero pad rows per 3-graph block."""
    rp = np.zeros((RPAD, F), np.float32)
    full = (GC * N) // (GPB * N)          # 21 full blocks
    rp[:full * NB] = sh[:full * GPB * N].reshape(full * NB, F)
    rem = GC * N - full * GPB * N
    if rem:
        rp[full * NB:full * NB + rem] = sh[full * GPB * N:]
    return rp


def _prep_stats_inputs(node_feats):
    nf = np.ascontiguousarray(node_feats, np.float32).reshape(G, N, F)
    maps = []
    for c in range(N_CORES):
        rp = _pad_rows(nf[c * GC:(c + 1) * GC].reshape(GC * N, F))
        nft = np.zeros((FPAD, RPAD), np.float32)
        nft[:F, :] = rp.T
        maps.append({"nft": nft})
    return maps


def _prep_main_inputs(inputs, node_feats, A, conv_w, conv_b, w1s, b1s,
                      W2, b2v, W3, b3v, W4, b4v,
                      cls_w1, cls_b1, prelu_a, cls_w2, cls_b2):
    f32 = lambda x: np.ascontiguousarray(x, np.float32)
    R = _round_f32r
    inputs, node_feats, A = f32(inputs), f32(node_feats), f32(A)
    cwbd = np.zeros((128, 32), np.float32)
    cb32 = np.zeros((32, 1), np.float32)
    wt = f32(conv_w).T  # [32, 6]
    for b in range(B):
        cwbd[32 * b:32 * (b + 1), 6 * b:6 * (b + 1)] = wt
        cb32[6 * b:6 * (b + 1), 0] = f32(conv_b)
    w1xp = np.zeros((FPAD, 512), np.float32)
    w1ap = np.zeros((FPAD, 512), np.float32)
    w1xp[:F] = w1s[:F]
    w1ap[:F] = w1s[F:]
    w1xp[F] = b1s          # bias via the constant-1 pad feature channel
    shared = {
        "w1x": R(w1xp), "w1a": R(w1ap),
        "w2x": R(W2[:512]), "w2a": R(W2[512:]),
        "w3x": R(W3[:256]), "w3a": R(W3[256:]),
        "w4x": R(W4[:128]), "w4a": R(W4[128:]),
        "b1": R(f32(b1s).reshape(1, 512)), "b2": R(f32(b2v).reshape(1, 256)),
        "b3": R(f32(b3v).reshape(1, 128)), "b4": f32(b4v).reshape(64, 1),
        "cw1": R(cls_w1), "cb1": f32(cls_b1).reshape(32, 1),
        "pa": f32(prelu_a).reshape(32, 1), "cw2": f32(cls_w2),
        "cb2": f32(cls_b2).reshape(2, 1),
        "cwbd": R(cwbd), "cb32": cb32,
        "ones": np.ones((1, NB), np.float32),
    }
    # atbi: [123, NBLK*256]: per block [blockdiag(A_g^T) | I | 0]
    At = A.transpose(0, 2, 1)             # [G, 41, 41] = A^T per graph
    eye = np.eye(NB, dtype=np.float32)
    maps = []
    for c in range(N_CORES):
        img = inputs[:, :, c * HS:(c + 1) * HS, :].reshape(128, NPX)
        rp = _pad_rows(node_feats[c * GC:(c + 1) * GC].reshape(GC * N, F))
        x0r = np.zeros((NB, NBLK, FPAD), np.float32)
        x0r[:, :, :F] = rp.reshape(NBLK, NB, F).transpose(1, 0, 2)
        x0r[:, :, F] = 1.0   # constant-1 feature carrying b1 through bmm1
        atbi = np.zeros((NB, NBLK, ATW), np.float32)
        Ash = At[c * GC:(c + 1) * GC]
        for b in range(NBLK):
            ng = min(GPB, GC - b * GPB)
            for gi in range(ng):
                atbi[N * gi:N * (gi + 1), b, N * gi:N * (gi + 1)] = \
                    Ash[b * GPB + gi]
            atbi[:, b, NB:2 * NB] = eye
        m = dict(shared)
        m.update({"img": R(img), "x0r": R(x0r.reshape(NB, NBLK * FPAD)),
                  "atbi": R(atbi.reshape(NB, NBLK * ATW))})
        maps.append(m)
    return maps


# ------------------------------------------------------------- execution
def _output_names(nc):
    names = []
    for alloc in nc.m.functions[0].allocations:
        if isinstance(alloc, mybir.MemoryLocationSet) and \
                alloc.kind == "ExternalOutput":
            names.append(alloc.memorylocations[0].name)
    return names


def _run(nc, in_maps):
    if os.environ.get("KERNEL_SIM"):
        from concourse.bass_interp import MultiCoreSim
        sim = MultiCoreSim(nc, num_cores=N_CORES, trace=False)
        for c in range(N_CORES):
            for k, v in in_maps[c].items():
                sim.cores[c].tensor(k)[:] = v
        sim.simulate()
        onames = _output_names(nc)
        return [{n: np.array(sim.cores[c].tensor(n)) for n in onames}
                for c in range(N_CORES)]
    from concourse.bass_utils import run_bass_kernel_spmd
    return run_bass_kernel_spmd(nc, in_maps, list(range(N_CORES))).results


def _get_nc(which, reps=1):
    key = (which, reps)
    with _lock:
        if key not in _cache:
            if which == "stats":
                _cache[key] = _build_stats_nc()
            else:
                _cache[key] = _build_main_nc(reps)
        return _cache[key]


def kernel(inputs, node_feats, A, knn_inx, conv_w, conv_b, W1, b1, W2, b2,
           W3, b3, W4, b4, cls_w1, cls_b1, prelu_a, cls_w2, cls_b2):
    f32 = lambda x: np.asarray(x, np.float32)
    # ---- dispatch A: BN statistics ------------------------------------
    nc_s = _get_nc("stats")
    smaps = _prep_stats_inputs(node_feats)
    sres = _run(nc_s, smaps)
    tot = np.zeros((FPAD, 2), np.float64)
    for c in range(N_CORES):
        tot += sres[c]["stats"].astype(np.float64)
    cnt = float(G * N)
    mean = (tot[:F, 0] / cnt).astype(np.float32)
    var = (tot[:F, 1] / cnt).astype(np.float32) - mean * mean
    s = (1.0 / np.sqrt(var + np.float32(EPS))).astype(np.float32)
    t_ = (-mean * s).astype(np.float32)
    W1 = f32(W1)
    sdup = np.concatenate([s, s]).astype(np.float32)
    w1s = W1 * sdup[:, None]
    b1s = f32(b1) + t_ @ W1[:F] + t_ @ W1[F:]

    # ---- dispatch B: conv + GCN ---------------------------------------
    nc_m = _get_nc("main")
    mmaps = _prep_main_inputs(inputs, node_feats, A, conv_w, conv_b,
                              w1s, b1s, W2, b2, W3, b3, W4, b4,
                              cls_w1, cls_b1, prelu_a, cls_w2, cls_b2)
    mres = _run(nc_m, mmaps)

    # ---- host gather/unshard ------------------------------------------
    pred_maps = np.concatenate(
        [mres[c]["pred"] for c in range(N_CORES)], axis=2)
    x4pred = np.zeros((G * N, 2), np.float32)
    for c in range(N_CORES):
        pt = mres[c]["predt"]  # [2, RPAD]
        sh = pt.T.reshape(NBLK, NB, 2)
        full = (GC * N) // (GPB * N)
        rem = GC * N - full * GPB * N
        rows = [sh[:full].reshape(full * NB, 2)]
        if rem:
            rows.append(sh[full, :rem])
        x4pred[c * GC * N:(c + 1) * GC * N] = np.concatenate(rows, 0)
    x4pred = x4pred.reshape(G, N, 2)
    ki = np.asarray(knn_inx)
    gcn_pred = np.take_along_axis(
        x4pred, ki[:, :, None].astype(np.int64), axis=1).reshape(-1, 2)
    return pred_maps, gcn_pred
